# revision 2
# baseline (speedup 1.0000x reference)
"""Trainium2 Bass kernel for nn_KinematicOperation (kinematic tree forward).

Structure of the (deterministic) problem instance:
  - N = 1 + 2048*768 + 2048*256 atoms.
  - gen0: 2048 chains of 768 atoms rooted at the virtual root (identity HT);
    chain atoms are contiguous: chain c = atoms [1+c*768, 1+(c+1)*768).
  - gen1: 2048 branches of 256 atoms rooted mid-chain (gen0 chain c position
    384); branch atoms contiguous starting at boff = 1 + 2048*768.
  - Local HTs: BOND everywhere except a JUMP at each chain start; root = I.
  - Output: coords[id_idx[a-1]] = prefix_HT(a)[:3, 3] for atoms a = 1..N-1.

Sharding: core k owns gen0 chains [256k, 256(k+1)) and gen1 branches of the
same index range, so the branch-root HT handoff between generations stays
on-core and no collectives are needed.  Host pre-slices bond dof columns
(0..3 of 9) and gathers jump rows, shrinking input DMA.

v2: dual-engine execution.  The lane-parallel phases (bond fold, level-1
scan, w, cumsum, down-transform) are split between the DVE (Vector) and the
GPSIMD (Pool) engine by contiguous block-lane ranges; each engine owns
private X/w/scratch tiles for its lanes.  The scalar engine (ACT) computes
trig and the copy/negate fold entries; GPSIMD also does the prefill memsets,
the jump-rotation build, and gen1's level-2 composes.  The serial hierarchy
(levels, excl) stays on DVE over shared tiles fed by both engines' bht
parts.

Device algorithm per generation (fp32; rotations stored as 3x3 row-major,
translations separately):
  - 3-level blocked prefix scan along each chain:
      level1: rotation-only scan propagating ROWS 0,1 (6 elems) in place;
      translations via the NeRF identity local_t = d * col0(localR):
      t_glob(p) = sum_{q<=p} d_q * col0(R_glob_q), so in-block translations
      are prefix SUMS of w = d * col0(R_inblock) (col0 z-comp from a cross
      product), then level2/3 compose full 3x4 block HTs (tiny), and the
      final transform applies block-exclusive R,t to the in-block cumsums.
  - Output xyz written scatter-ready; host applies the id_idx permutation.
"""

import os
import sys

import numpy as np

for _p in ("/opt/trn_rl_repo", "/root/.axon_site/_ro/trn_rl_repo"):
    if os.path.isdir(_p) and _p not in sys.path:
        sys.path.insert(0, _p)

# ---------------------------------------------------------------- constants
C0, L0 = 2048, 768
C1, L1 = 2048, 256
N = 1 + C0 * L0 + C1 * L1
BOFF = 1 + C0 * L0
NCORES = 8
P = 128
CHI = 2                      # chains per partition (256 chains per core)
CH0 = C0 // NCORES
CH1 = C1 // NCORES
A0 = CH0 * L0                # 196608 gen0 atoms per core
A1 = CH1 * L1                # 65536 gen1 atoms per core

# block geometry: L = T*J,  J = S*U supers x blocks
T0, J0, S0, U0 = 12, 64, 8, 8
F0 = CHI * J0                # 128 block-lanes per partition
T1, J1, S1, U1 = 8, 32, 4, 8
F1 = CHI * J1                # 64

# engine lane split (DVE gets [0, ND), gpsimd gets [ND, F))
ND0 = 84
NG0 = F0 - ND0               # 44
ND1 = 44
NG1 = F1 - ND1               # 20

PI = float(np.pi)

_CACHE = {}


# ------------------------------------------------------------- device build
def _build_program(repeat=1):
    from concourse import bacc, mybir, tile
    from concourse.bass import AP

    f32 = mybir.dt.float32
    SIN = mybir.ActivationFunctionType.Sin
    ABS = mybir.ActivationFunctionType.Abs
    CPY = mybir.ActivationFunctionType.Copy

    nc = bacc.Bacc("TRN2", target_bir_lowering=False, debug=False)

    b0_d = nc.dram_tensor("b0", [A0, 4], f32, kind="ExternalInput")
    th0_d = nc.dram_tensor("th0", [A0], f32, kind="ExternalInput")
    b1_d = nc.dram_tensor("b1", [A1, 4], f32, kind="ExternalInput")
    jd_d = nc.dram_tensor("jd", [P, CHI * 9], f32, kind="ExternalInput")
    kin0_d = nc.dram_tensor("kin0", [P, F0 * T0 * 3], f32, kind="ExternalOutput")
    kin1_d = nc.dram_tensor("kin1", [P, F1 * T1 * 3], f32, kind="ExternalOutput")

    def apx(tl, off, *dims):
        """AP over tile-AP `tl` at free-elem offset `off` with free dims
        [(step, count), ...] (full 128 partitions)."""
        t = tl[:] if not isinstance(tl, AP) else tl
        return AP(t.tensor, t.offset + off, [[t.ap[0][0], P]] + [list(d) for d in dims])

    def compose_1d(E, lanes, a_off, a_step, b_off, b_step, o_off, o_step,
                   tA, tB, a_tile, b_tile, o_tile):
        """C = A @ B (3x4 HT compose, 12-elem row-major layout) over lanes."""
        for k, dst in ((0, tA), (1, tB)):
            E.tensor_mul(
                out=apx(dst, 0, (12, lanes), (4, 3), (1, 4)),
                in0=apx(a_tile, a_off + k, (a_step, lanes), (4, 3), (0, 4)),
                in1=apx(b_tile, b_off + 4 * k, (b_step, lanes), (0, 3), (1, 4)),
            )
        E.tensor_add(
            out=apx(tA, 0, (12, lanes), (1, 12)),
            in0=apx(tA, 0, (12, lanes), (1, 12)),
            in1=apx(tB, 0, (12, lanes), (1, 12)))
        E.tensor_mul(
            out=apx(tB, 0, (12, lanes), (4, 3), (1, 4)),
            in0=apx(a_tile, a_off + 2, (a_step, lanes), (4, 3), (0, 4)),
            in1=apx(b_tile, b_off + 8, (b_step, lanes), (0, 3), (1, 4)),
        )
        E.tensor_add(
            out=apx(o_tile, o_off, (o_step, lanes), (1, 12)),
            in0=apx(tA, 0, (12, lanes), (1, 12)),
            in1=apx(tB, 0, (12, lanes), (1, 12)),
        )
        E.tensor_add(
            out=apx(o_tile, o_off + 3, (o_step, lanes), (4, 3)),
            in0=apx(o_tile, o_off + 3, (o_step, lanes), (4, 3)),
            in1=apx(a_tile, a_off + 3, (a_step, lanes), (4, 3)),
        )

    def excl_blocks(E, SC, CS, U, LPS, spx, lp2, rx, tA, tB):
        """rx[cs, u] = spx[cs] @ lp2[cs, u]  (exclusive block prefixes);
        u=0 is spx itself (lp2[cs,0] == I), copied on ACT."""
        SC.copy(out=apx(rx, 0, (U * 12, CS), (1, 12)),
                in_=apx(spx, 0, (12, CS), (1, 12)))
        UM = U - 1
        for i in range(3):
            for k, dst in ((0, tA), (1, tB)):
                E.tensor_mul(
                    out=apx(dst, 4 * i, (96, CS), (12, UM), (1, 4)),
                    in0=apx(spx, 4 * i + k, (12, CS), (0, UM), (0, 4)),
                    in1=apx(lp2, 12 + 4 * k, (LPS, CS), (12, UM), (1, 4)))
            E.tensor_add(
                out=apx(tA, 4 * i, (96, CS), (12, UM), (1, 4)),
                in0=apx(tA, 4 * i, (96, CS), (12, UM), (1, 4)),
                in1=apx(tB, 4 * i, (96, CS), (12, UM), (1, 4)))
            E.tensor_mul(
                out=apx(tB, 4 * i, (96, CS), (12, UM), (1, 4)),
                in0=apx(spx, 4 * i + 2, (12, CS), (0, UM), (0, 4)),
                in1=apx(lp2, 12 + 8, (LPS, CS), (12, UM), (1, 4)))
            E.tensor_add(
                out=apx(rx, 12 + 4 * i, (96, CS), (12, UM), (1, 4)),
                in0=apx(tA, 4 * i, (96, CS), (12, UM), (1, 4)),
                in1=apx(tB, 4 * i, (96, CS), (12, UM), (1, 4)))
        E.tensor_add(
            out=apx(rx, 12 + 3, (96, CS), (12, UM), (4, 3)),
            in0=apx(rx, 12 + 3, (96, CS), (12, UM), (4, 3)),
            in1=apx(spx, 3, (12, CS), (0, UM), (4, 3)))

    # ---- per-engine-region emitters.  Each engine owns a private X/w tile
    # for lanes [f0, f0+nfp) of a generation; shared tiles (dof, trig, bht,
    # rx) are indexed with the global lane offset f0.

    def emit_trig_fold(V, SC, dof, trig, L, halfpi, alpha_fix,
                      theta_src=None):
        """Angle-folded trig: alpha_p = phi_c[p-1] + phi_p[p]; planes
        sa/ca = sin/cos(alpha), st/ct = sin/cos(theta).  One DVE wrap per
        angle (cos plane as scratch), cos = sin(pi/2 - |w|) on ACT."""
        apl, aw = trig["apl"], trig["aw"]

        def one(src, cosn, sinn):
            V.add_range_wrap(out=trig[cosn][:], in_=src, shift=0.0,
                             bound=PI, period=2 * PI)
            SC.activation(out=trig[sinn][:], in_=trig[cosn][:], func=SIN)
            SC.activation(out=aw[:], in_=trig[cosn][:], func=ABS)
            SC.activation(out=trig[cosn][:], in_=aw[:], func=SIN,
                          scale=-1.0, bias=halfpi[:])

        if theta_src is None:
            theta_src = apx(dof, 1, (L * 4, CHI), (4, L))
        one(theta_src, "ct", "st")
        V.tensor_add(out=apx(apl, 1, (L, CHI), (1, L - 1)),
                     in0=apx(dof, 4, (L * 4, CHI), (4, L - 1)),
                     in1=apx(dof, 3, (L * 4, CHI), (4, L - 1)))
        alpha_fix(apl)
        one(apx(apl, 0, (L, CHI), (1, L)), "ca", "sa")

    def emit_fold(E, SC, trig, X, T, nfp, f0):
        """Folded local factor L' = Rx(alpha)Rz(pi-theta) for lane range
        [f0, f0+nfp) into private X:
        [[-ct, -st, 0], [ca*st, -ca*ct, -sa], [sa*st, -sa*ct, ca]].
        Copy/negate entries go to ACT; products stay on E."""
        def tp(nm):
            return apx(trig[nm], f0 * T, (T, nfp), (1, T))

        def xo(e):
            return apx(X, e, (9, nfp), (nfp * 9, T))

        SC.activation(out=xo(0), in_=tp("ct"), func=CPY, scale=-1.0)
        SC.activation(out=xo(1), in_=tp("st"), func=CPY, scale=-1.0)
        SC.activation(out=xo(5), in_=tp("sa"), func=CPY, scale=-1.0)
        SC.activation(out=xo(8), in_=tp("ca"), func=CPY)
        E.tensor_mul(out=xo(3), in0=tp("ca"), in1=tp("st"))
        E.tensor_mul(out=xo(4), in0=tp("ca"), in1=xo(0))
        E.tensor_mul(out=xo(6), in0=tp("sa"), in1=tp("st"))
        E.tensor_mul(out=xo(7), in0=tp("sa"), in1=xo(0))

    def scan_step_instr(E, X, tA, tB, tC, nfp, t, i):
        """Instruction i (0..5) of zero-col scan step t over the private X
        (in-place state rows 0,1 in elems 0..5; local row2 in 6..8)."""
        pb = (t - 1) * nfp * 9
        cb = t * nfp * 9
        if i == 0:
            E.tensor_mul(out=apx(tA, 0, (6, nfp), (3, 2), (1, 2)),
                         in0=apx(X, pb + 0, (9, nfp), (3, 2), (0, 2)),
                         in1=apx(X, cb + 0, (9, nfp), (0, 2), (1, 2)))
        elif i == 1:
            E.tensor_mul(out=apx(tB, 0, (6, nfp), (3, 2), (1, 3)),
                         in0=apx(X, pb + 1, (9, nfp), (3, 2), (0, 3)),
                         in1=apx(X, cb + 3, (9, nfp), (0, 2), (1, 3)))
        elif i == 2:
            E.tensor_mul(out=apx(tC, 0, (6, nfp), (3, 2), (1, 3)),
                         in0=apx(X, pb + 2, (9, nfp), (3, 2), (0, 3)),
                         in1=apx(X, cb + 6, (9, nfp), (0, 2), (1, 3)))
        elif i == 3:
            E.tensor_add(out=apx(tA, 0, (6, nfp), (3, 2), (1, 2)),
                         in0=apx(tA, 0, (6, nfp), (3, 2), (1, 2)),
                         in1=apx(tB, 0, (6, nfp), (3, 2), (1, 2)))
        elif i == 4:
            E.tensor_add(out=apx(X, cb, (9, nfp), (3, 2), (1, 2)),
                         in0=apx(tA, 0, (6, nfp), (3, 2), (1, 2)),
                         in1=apx(tC, 0, (6, nfp), (3, 2), (1, 2)))
        else:
            E.tensor_add(out=apx(X, cb + 2, (9, nfp), (3, 2)),
                         in0=apx(tB, 2, (6, nfp), (3, 2)),
                         in1=apx(tC, 2, (6, nfp), (3, 2)))

    def emit_w_i(E, X, w, dof, tA, tB, T, nfp, f0, i):
        """Instruction i (0..4) of the w-phase: w = d * col0(R_inblock);
        R20 via cross product kept in tA[t*nfp + f]."""
        d_ap = apx(dof, f0 * T * 4 + 2, (T * 4, nfp), (4, T))
        if i == 0:
            E.tensor_mul(out=apx(tA, 0, (nfp, T), (1, nfp)),
                         in0=apx(X, 1, (nfp * 9, T), (9, nfp)),
                         in1=apx(X, 5, (nfp * 9, T), (9, nfp)))
        elif i == 1:
            E.tensor_mul(out=apx(tB, 0, (nfp, T), (1, nfp)),
                         in0=apx(X, 2, (nfp * 9, T), (9, nfp)),
                         in1=apx(X, 4, (nfp * 9, T), (9, nfp)))
        elif i == 2:
            E.tensor_sub(out=apx(tA, 0, (nfp, T), (1, nfp)),
                         in0=apx(tA, 0, (nfp, T), (1, nfp)),
                         in1=apx(tB, 0, (nfp, T), (1, nfp)))
        elif i == 3:
            E.tensor_mul(out=apx(w, 2, (3, nfp), (nfp * 3, T)),
                         in0=apx(tA, 0, (1, nfp), (nfp, T)),
                         in1=d_ap)
        else:
            E.tensor_mul(out=apx(w, 0, (3, nfp), (nfp * 3, T), (1, 2)),
                         in0=apx(X, 0, (9, nfp), (nfp * 9, T), (3, 2)),
                         in1=apx(dof, f0 * T * 4 + 2, (T * 4, nfp), (4, T),
                                 (0, 2)))

    def emit_bht(E, SC, X, w, bht, tA, tB, T, nfp, f0):
        """Assemble 12-elem (3x4 row-major) block-total HTs for lanes
        [f0, f0+nfp) from the private scan state at slab T-1 into the
        shared bht tile (r20 reused from tA's w-phase cross products)."""
        base = (T - 1) * nfp * 9
        SC.copy(out=apx(bht, f0 * 12, (12, nfp), (4, 2), (1, 3)),
                in_=apx(X, base, (9, nfp), (3, 2), (1, 3)))
        SC.copy(out=apx(bht, f0 * 12 + 8, (12, nfp)),
                in_=apx(tA, (T - 1) * nfp, (1, nfp)))
        # r21 = r02*r10 - r00*r12 ; r22 = r00*r11 - r01*r10
        for dst, (i1, i2), (i3, i4) in ((9, (2, 3), (0, 5)),
                                        (10, (0, 4), (1, 3))):
            E.tensor_mul(out=apx(tA, 0, (1, nfp)),
                         in0=apx(X, base + i1, (9, nfp)),
                         in1=apx(X, base + i2, (9, nfp)))
            E.tensor_mul(out=apx(tB, 0, (1, nfp)),
                         in0=apx(X, base + i3, (9, nfp)),
                         in1=apx(X, base + i4, (9, nfp)))
            E.tensor_sub(out=apx(bht, f0 * 12 + dst, (12, nfp)),
                         in0=apx(tA, 0, (1, nfp)),
                         in1=apx(tB, 0, (1, nfp)))
        SC.copy(out=apx(bht, f0 * 12 + 3, (12, nfp), (4, 3)),
                in_=apx(w, (T - 1) * nfp * 3, (3, nfp), (1, 3)))

    def emit_down_i(E, w, rx, X, xoff, tmpoff, T, nfp, f0, i):
        """Instruction i (0..5) of the down-transform for lanes
        [f0, f0+nfp): xyz[f, t, c] = (R_bexcl @ w_cum)[c] + t_bexcl[c]."""
        xyz = apx(X, xoff, (T * 3, nfp), (3, T), (1, 3))
        tmp = apx(X, tmpoff, (T * 3, nfp), (3, T), (1, 3))

        def rxk(k):
            return apx(rx, f0 * 12 + k, (12, nfp), (0, T), (4, 3))

        def wk(k):
            return apx(w, k, (3, nfp), (nfp * 3, T), (0, 3))

        if i == 0:
            E.tensor_mul(out=xyz, in0=rxk(0), in1=wk(0))
        elif i == 1:
            E.tensor_mul(out=tmp, in0=rxk(1), in1=wk(1))
        elif i == 2:
            E.tensor_add(out=xyz, in0=xyz, in1=tmp)
        elif i == 3:
            E.tensor_mul(out=tmp, in0=rxk(2), in1=wk(2))
        elif i == 4:
            E.tensor_add(out=xyz, in0=xyz, in1=tmp)
        else:
            E.tensor_add(out=xyz, in0=xyz, in1=rxk(3))

    with tile.TileContext(nc) as tc:
      for _rep in range(repeat):
        with tc.tile_pool(name="main", bufs=1) as mp:
            dof0 = mp.tile([P, CHI * L0 * 4], f32)
            dof1 = mp.tile([P, CHI * L1 * 4], f32)
            th0 = mp.tile([P, CHI * L0], f32)
            X0d = mp.tile([P, T0 * ND0 * 9], f32)
            X0g = mp.tile([P, T0 * NG0 * 9], f32)
            w0d = mp.tile([P, T0 * ND0 * 3], f32)
            w0g = mp.tile([P, T0 * NG0 * 3], f32)
            tAh = mp.tile([P, 96 * CHI * S0], f32)     # DVE scratch (scan/w/hier)
            tBh = mp.tile([P, 96 * CHI * S0], f32)
            tC0d = mp.tile([P, 6 * ND0], f32)
            tA0g = mp.tile([P, max(12 * NG0, 96 * 6)], f32)
            tB0g = mp.tile([P, max(12 * NG0, 96 * 6)], f32)
            tC0g = mp.tile([P, 6 * NG0], f32)
            rx0 = mp.tile([P, F0 * 12], f32)
            bht0 = mp.tile([P, F0 * 12], f32)
            lp2_0 = mp.tile([P, CHI * S0 * (U0 + 1) * 12], f32)
            spx0 = mp.tile([P, CHI * S0 * 12], f32)
            lp2_1 = mp.tile([P, CHI * S1 * (U1 + 1) * 12], f32)
            spx1 = mp.tile([P, CHI * S1 * 12], f32)
            rbr = mp.tile([P, CHI * 12], f32)
            a32 = mp.tile([P, CHI * 12], f32)
            jd = mp.tile([P, CHI * 9], f32)
            jang = mp.tile([P, CHI * 2 * 3], f32)
            jsin = mp.tile([P, CHI * 2 * 3], f32)
            jcos = mp.tile([P, CHI * 2 * 3], f32)
            re_ = mp.tile([P, CHI * 2 * 9], f32)
            rj = mp.tile([P, CHI * 9], f32)
            jtmp = mp.tile([P, CHI * 2 * 9], f32)
            halfpi = mp.tile([P, 1], f32)
            trig1 = {nm: mp.tile([P, CHI * L1], f32, name=f"t1_{nm}")
                     for nm in ("sa", "ca", "st", "ct", "apl", "aw")}

            V = nc.vector
            SC = nc.scalar
            G = nc.gpsimd

            nc.sync.dma_start(out=jd[:], in_=jd_d[:])
            V.memset(halfpi[:], PI / 2)

            src = AP(th0_d, 0, [[L0, P], [P * L0, CHI], [1, L0]])
            dst = AP(th0[:].tensor, th0[:].offset,
                     [[th0[:].ap[0][0], P], [L0, CHI], [1, L0]])
            nc.sync.dma_start(out=dst, in_=src)
            src = AP(b0_d, 0, [[L0 * 4, P], [P * L0 * 4, CHI], [1, L0 * 4]])
            dst = AP(dof0[:].tensor, dof0[:].offset,
                     [[dof0[:].ap[0][0], P], [L0 * 4, CHI], [1, L0 * 4]])
            nc.sync.dma_start(out=dst, in_=src)
            src = AP(b1_d, 0, [[L1 * 4, P], [P * L1 * 4, CHI], [1, L1 * 4]])
            dst = AP(dof1[:].tensor, dof1[:].offset,
                     [[dof1[:].ap[0][0], P], [L1 * 4, CHI], [1, L1 * 4]])
            nc.sync.dma_start(out=dst, in_=src)

            # ---- prefill on gpsimd (runs in the input-DMA wait window) ----
            G.memset(lp2_0[:], 0.0)
            G.memset(apx(lp2_0, 0, ((U0 + 1) * 12, CHI * S0), (5, 3)), 1.0)
            G.memset(spx0[:], 0.0)
            G.memset(apx(spx0, 0, (S0 * 12, CHI), (5, 3)), 1.0)
            G.memset(lp2_1[:], 0.0)
            G.memset(apx(lp2_1, 0, ((U1 + 1) * 12, CHI * S1), (5, 3)), 1.0)
            G.memset(apx(X0g, 2, (9, NG0)), 0.0)
            V.memset(apx(X0d, 2, (9, ND0)), 0.0)

            # ---- JUMP HT build: wraps on DVE (tiny), sins on ACT,
            # rotation products on gpsimd ----
            V.tensor_copy(out=jang[:], in_=apx(jd, 3, (9, CHI), (3, 2),
                                               (1, 3)))
            V.add_range_wrap(out=jsin[:], in_=jang[:], shift=0.0,
                             bound=PI, period=2 * PI)
            SC.activation(out=jsin[:], in_=jsin[:], func=SIN)
            V.add_range_wrap(out=jcos[:], in_=jang[:], shift=PI / 2,
                             bound=PI, period=2 * PI)
            SC.activation(out=jcos[:], in_=jcos[:], func=SIN)

            CR = CHI * 2

            def sc_(tl, ang):
                return apx(tl, ang, (3, CR))

            def re(e):
                return apx(re_, e, (9, CR))

            def jt1(e):
                return apx(jtmp, e, (9, CR))

            sa = lambda: sc_(jsin, 0)
            sb = lambda: sc_(jsin, 1)
            s_c = lambda: sc_(jsin, 2)
            ca = lambda: sc_(jcos, 0)
            cb = lambda: sc_(jcos, 1)
            c_c = lambda: sc_(jcos, 2)
            # R = Rz(c)Ry(b)Rx(a) per (chi, rot) lane
            G.tensor_mul(out=re(0), in0=c_c(), in1=cb())
            G.tensor_mul(out=jt1(0), in0=sb(), in1=sa())
            G.tensor_mul(out=jt1(1), in0=sb(), in1=ca())
            G.tensor_mul(out=jt1(2), in0=c_c(), in1=jt1(0))
            G.tensor_mul(out=jt1(3), in0=s_c(), in1=ca())
            G.tensor_sub(out=re(1), in0=jt1(2), in1=jt1(3))
            G.tensor_mul(out=jt1(2), in0=c_c(), in1=jt1(1))
            G.tensor_mul(out=jt1(3), in0=s_c(), in1=sa())
            G.tensor_add(out=re(2), in0=jt1(2), in1=jt1(3))
            G.tensor_mul(out=re(3), in0=s_c(), in1=cb())
            G.tensor_mul(out=jt1(2), in0=s_c(), in1=jt1(0))
            G.tensor_mul(out=jt1(3), in0=c_c(), in1=ca())
            G.tensor_add(out=re(4), in0=jt1(2), in1=jt1(3))
            G.tensor_mul(out=jt1(2), in0=s_c(), in1=jt1(1))
            G.tensor_mul(out=jt1(3), in0=c_c(), in1=sa())
            G.tensor_sub(out=re(5), in0=jt1(2), in1=jt1(3))
            G.tensor_scalar_mul(out=re(6), in0=sb(), scalar1=-1.0)
            G.tensor_mul(out=re(7), in0=cb(), in1=sa())
            G.tensor_mul(out=re(8), in0=cb(), in1=ca())
            # rj = R1 @ R2 (3x3), lanes = chi
            G.tensor_mul(
                out=apx(rj, 0, (9, CHI), (3, 3), (1, 3)),
                in0=apx(re_, 0, (18, CHI), (3, 3), (0, 3)),
                in1=apx(re_, 9, (18, CHI), (0, 3), (1, 3)))
            G.tensor_mul(
                out=apx(jtmp, 0, (9, CHI), (3, 3), (1, 3)),
                in0=apx(re_, 1, (18, CHI), (3, 3), (0, 3)),
                in1=apx(re_, 12, (18, CHI), (0, 3), (1, 3)))
            G.tensor_add(out=rj[:, : CHI * 9], in0=rj[:, : CHI * 9],
                         in1=jtmp[:, : CHI * 9])
            G.tensor_mul(
                out=apx(jtmp, 0, (9, CHI), (3, 3), (1, 3)),
                in0=apx(re_, 2, (18, CHI), (3, 3), (0, 3)),
                in1=apx(re_, 15, (18, CHI), (0, 3), (1, 3)))
            G.tensor_add(out=rj[:, : CHI * 9], in0=rj[:, : CHI * 9],
                         in1=jtmp[:, : CHI * 9])

            # ================= GEN 0: trig + folds =================
            with tc.tile_pool(name="ptrig0", bufs=1) as pt:
                trig0 = {nm: pt.tile([P, CHI * L0], f32, name=f"t0_{nm}")
                         for nm in ("sa", "ca", "st", "ct", "apl", "aw")}

                def afix0(apl):
                    # chain position 1 has the jump as parent: alpha = phi_p
                    V.tensor_copy(out=apx(apl, 1, (L0, CHI)),
                                  in_=apx(dof0, 4, (L0 * 4, CHI)))

                emit_trig_fold(V, SC, dof0, trig0, L0, halfpi, afix0,
                               theta_src=th0[:])
                emit_fold(V, SC, trig0, X0d, T0, ND0, 0)
                emit_fold(G, SC, trig0, X0g, T0, NG0, ND0)

                # full jump 3x3 -> X0d slab 0, lanes f = chi*J0 (j=0)
                V.tensor_copy(out=apx(X0d, 0, (J0 * 9, CHI), (1, 9)),
                              in_=apx(rj, 0, (9, CHI), (1, 9)))

                # gen1 trig early: ACT computes it under the gen0 scan
                def afix1(apl):
                    # branch position 0: alpha = phi_p + phi_c(gen0 atom 384)
                    V.tensor_add(out=apx(apl, 0, (L1, CHI)),
                                 in0=apx(dof1, 0, (L1 * 4, CHI)),
                                 in1=apx(dof0, 384 * 4 + 3, (L0 * 4, CHI)))

                emit_trig_fold(V, SC, dof1, trig1, L1, halfpi, afix1)

            # ======== level-1 scans + rest ========
            with tc.tile_pool(name="px1", bufs=1) as px:
                X1d = px.tile([P, T1 * ND1 * 9], f32)
                X1g = px.tile([P, T1 * NG1 * 9], f32)
                w1d = px.tile([P, T1 * ND1 * 3], f32)
                w1g = px.tile([P, T1 * NG1 * 3], f32)
                bht1 = px.tile([P, F1 * 12], f32)
                rx1 = px.tile([P, F1 * 12], f32)
                tA1d = px.tile([P, max(8 * ND1, 96 * CHI * S1)], f32)
                tB1d = px.tile([P, max(8 * ND1, 96 * CHI * S1)], f32)
                tC1d = px.tile([P, 6 * ND1], f32)
                tA1g = px.tile([P, 12 * NG1], f32)
                tB1g = px.tile([P, 12 * NG1], f32)
                tC1g = px.tile([P, 6 * NG1], f32)

                V.memset(apx(X1d, 2, (9, ND1)), 0.0)
                G.memset(apx(X1g, 2, (9, NG1)), 0.0)
                emit_fold(V, SC, trig1, X1d, T1, ND1, 0)
                emit_fold(G, SC, trig1, X1g, T1, NG1, ND1)

                # interleave the two gens' scan recurrences per engine
                for t in range(1, T0):
                    for i in range(6):
                        scan_step_instr(V, X0d, tAh, tBh, tC0d, ND0, t, i)
                        if t < T1:
                            scan_step_instr(V, X1d, tA1d, tB1d, tC1d, ND1,
                                            t, i)
                for t in range(1, T0):
                    for i in range(6):
                        scan_step_instr(G, X0g, tA0g, tB0g, tC0g, NG0, t, i)
                        if t < T1:
                            scan_step_instr(G, X1g, tA1g, tB1g, tC1g, NG1,
                                            t, i)

                # w-phases
                for i in range(5):
                    emit_w_i(V, X0d, w0d, dof0, tAh, tBh, T0, ND0, 0, i)
                    emit_w_i(V, X1d, w1d, dof1, tA1d, tB1d, T1, ND1, 0, i)
                for i in range(5):
                    emit_w_i(G, X0g, w0g, dof0, tA0g, tB0g, T0, NG0, ND0, i)
                    emit_w_i(G, X1g, w1g, dof1, tA1g, tB1g, T1, NG1, ND1, i)

                # jump translation overwrites w at (t=0, j=0) lanes (DVE side)
                V.tensor_copy(out=apx(w0d, 0, (J0 * 3, CHI), (1, 3)),
                              in_=apx(jd, 0, (9, CHI), (1, 3)))

                # a32: in-block HT of the branch root (j=32, t=0) per chi:
                # chi0 -> lane 32 (DVE tile), chi1 -> lane 96 (gpsimd tile,
                # local lane 96 - ND0)
                gl = 96 - ND0
                for xt, wt, lo, chi in ((X0d, w0d, 32, 0), (X0g, w0g, gl, 1)):
                    V.tensor_copy(out=apx(a32, chi * 12, (1, 1), (4, 2),
                                          (1, 3)),
                                  in_=apx(xt, lo * 9, (1, 1), (3, 2),
                                          (1, 3)))
                    for dsti, (i1, i2), (i3, i4) in ((8, (1, 5), (2, 4)),
                                                     (9, (2, 3), (0, 5)),
                                                     (10, (0, 4), (1, 3))):
                        V.tensor_mul(out=apx(tC0d, 0, (1, 1)),
                                     in0=apx(xt, lo * 9 + i1, (1, 1)),
                                     in1=apx(xt, lo * 9 + i2, (1, 1)))
                        V.tensor_mul(out=apx(tC0d, 2, (1, 1)),
                                     in0=apx(xt, lo * 9 + i3, (1, 1)),
                                     in1=apx(xt, lo * 9 + i4, (1, 1)))
                        V.tensor_sub(out=apx(a32, chi * 12 + dsti, (1, 1)),
                                     in0=apx(tC0d, 0, (1, 1)),
                                     in1=apx(tC0d, 2, (1, 1)))

                # interleaved cumsums per engine
                for t in range(1, T0):
                    V.tensor_add(out=apx(w0d, t * ND0 * 3, (1, ND0 * 3)),
                                 in0=apx(w0d, t * ND0 * 3, (1, ND0 * 3)),
                                 in1=apx(w0d, (t - 1) * ND0 * 3,
                                         (1, ND0 * 3)))
                    if t < T1:
                        V.tensor_add(
                            out=apx(w1d, t * ND1 * 3, (1, ND1 * 3)),
                            in0=apx(w1d, t * ND1 * 3, (1, ND1 * 3)),
                            in1=apx(w1d, (t - 1) * ND1 * 3, (1, ND1 * 3)))
                for t in range(1, T0):
                    G.tensor_add(out=apx(w0g, t * NG0 * 3, (1, NG0 * 3)),
                                 in0=apx(w0g, t * NG0 * 3, (1, NG0 * 3)),
                                 in1=apx(w0g, (t - 1) * NG0 * 3,
                                         (1, NG0 * 3)))
                    if t < T1:
                        G.tensor_add(
                            out=apx(w1g, t * NG1 * 3, (1, NG1 * 3)),
                            in0=apx(w1g, t * NG1 * 3, (1, NG1 * 3)),
                            in1=apx(w1g, (t - 1) * NG1 * 3, (1, NG1 * 3)))

                # a32 translation part (after cumsum slab0 is final anyway)
                V.tensor_copy(out=apx(a32, 3, (1, 1), (4, 3)),
                              in_=apx(w0d, 32 * 3, (1, 1), (1, 3)))
                V.tensor_copy(out=apx(a32, 12 + 3, (1, 1), (4, 3)),
                              in_=apx(w0g, gl * 3, (1, 1), (1, 3)))

                # block-total HTs
                emit_bht(V, SC, X0d, w0d, bht0, tAh, tBh, T0, ND0, 0)
                emit_bht(V, SC, X1d, w1d, bht1, tA1d, tB1d, T1, ND1, 0)
                emit_bht(G, SC, X0g, w0g, bht0, tA0g, tB0g, T0, NG0, ND0)
                emit_bht(G, SC, X1g, w1g, bht1, tA1g, tB1g, T1, NG1, ND1)

                # ---- hierarchy: gen0 levels on DVE, gen1 level-2 on gpsimd
                LPS0 = (U0 + 1) * 12
                LPS1 = (U1 + 1) * 12
                SC.copy(out=apx(lp2_0, 12, (LPS0, CHI * S0), (1, 12)),
                        in_=apx(bht0, 0, (U0 * 12, CHI * S0), (1, 12)))
                SC.copy(out=apx(lp2_1, 12, (LPS1, CHI * S1), (1, 12)),
                        in_=apx(bht1, 0, (U1 * 12, CHI * S1), (1, 12)))
                for u in range(1, U0):
                    compose_1d(V, CHI * S0,
                               a_off=u * 12, a_step=LPS0,
                               b_off=u * 12, b_step=U0 * 12,
                               o_off=(u + 1) * 12, o_step=LPS0,
                               tA=tAh, tB=tBh,
                               a_tile=lp2_0, b_tile=bht0, o_tile=lp2_0)
                for u in range(1, U1):
                    compose_1d(G, CHI * S1,
                               a_off=u * 12, a_step=LPS1,
                               b_off=u * 12, b_step=U1 * 12,
                               o_off=(u + 1) * 12, o_step=LPS1,
                               tA=tA1g, tB=tB1g,
                               a_tile=lp2_1, b_tile=bht1, o_tile=lp2_1)
                for sidx in range(1, S0):
                    compose_1d(V, CHI,
                               a_off=(sidx - 1) * 12, a_step=S0 * 12,
                               b_off=(sidx - 1) * LPS0 + U0 * 12,
                               b_step=S0 * LPS0,
                               o_off=sidx * 12, o_step=S0 * 12,
                               tA=tAh, tB=tBh,
                               a_tile=spx0, b_tile=lp2_0, o_tile=spx0)
                excl_blocks(V, SC, CHI * S0, U0, LPS0, spx0, lp2_0, rx0,
                            tAh, tBh)
                compose_1d(V, CHI,
                           a_off=32 * 12, a_step=J0 * 12,
                           b_off=0, b_step=12,
                           o_off=0, o_step=12,
                           tA=tAh, tB=tBh,
                           a_tile=rx0, b_tile=a32, o_tile=rbr)
                # seed gen1 level-3 with the branch-root global HTs
                SC.copy(out=apx(spx1, 0, (S1 * 12, CHI), (1, 12)),
                        in_=apx(rbr, 0, (12, CHI), (1, 12)))

                # down-g0 on both engines; DVE's interleaved with the tiny
                # serial gen1 level-3 composes
                emit_down_i(V, w0d, rx0, X0d, 0, ND0 * T0 * 3, T0, ND0, 0, 0)
                for i in range(6):
                    emit_down_i(G, w0g, rx0, X0g, 0, NG0 * T0 * 3, T0, NG0,
                                ND0, i)
                for sidx in range(1, S1):
                    compose_1d(V, CHI,
                               a_off=(sidx - 1) * 12, a_step=S1 * 12,
                               b_off=(sidx - 1) * LPS1 + U1 * 12,
                               b_step=S1 * LPS1,
                               o_off=sidx * 12, o_step=S1 * 12,
                               tA=tA1d, tB=tB1d,
                               a_tile=spx1, b_tile=lp2_1, o_tile=spx1)
                    if sidx < 4:
                        emit_down_i(V, w0d, rx0, X0d, 0, ND0 * T0 * 3, T0,
                                    ND0, 0, sidx)
                excl_blocks(V, SC, CHI * S1, U1, LPS1, spx1, lp2_1, rx1,
                            tA1d, tB1d)
                emit_down_i(V, w0d, rx0, X0d, 0, ND0 * T0 * 3, T0, ND0, 0, 4)
                emit_down_i(V, w0d, rx0, X0d, 0, ND0 * T0 * 3, T0, ND0, 0, 5)
                nc.sync.dma_start(
                    out=AP(kin0_d, 0, [[F0 * T0 * 3, P], [1, ND0 * T0 * 3]]),
                    in_=apx(X0d, 0, (1, ND0 * T0 * 3)))
                nc.sync.dma_start(
                    out=AP(kin0_d, ND0 * T0 * 3,
                           [[F0 * T0 * 3, P], [1, NG0 * T0 * 3]]),
                    in_=apx(X0g, 0, (1, NG0 * T0 * 3)))
                for i in range(6):
                    emit_down_i(V, w1d, rx1, X1d, 0, ND1 * T1 * 3, T1, ND1,
                                0, i)
                for i in range(6):
                    emit_down_i(G, w1g, rx1, X1g, 0, NG1 * T1 * 3, T1, NG1,
                                ND1, i)
                nc.sync.dma_start(
                    out=AP(kin1_d, 0, [[F1 * T1 * 3, P], [1, ND1 * T1 * 3]]),
                    in_=apx(X1d, 0, (1, ND1 * T1 * 3)))
                nc.sync.dma_start(
                    out=AP(kin1_d, ND1 * T1 * 3,
                           [[F1 * T1 * 3, P], [1, NG1 * T1 * 3]]),
                    in_=apx(X1g, 0, (1, NG1 * T1 * 3)))

    nc.compile()
    return nc


def get_program(repeat=1):
    key = ("nc", repeat)
    if key not in _CACHE:
        _CACHE[key] = _build_program(repeat)
    return _CACHE[key]


# ------------------------------------------------------------------- host
def _shard_inputs(dofs, doftype):
    """Build the 8 per-core input maps (lane order (p, chi, j, t))."""
    in_maps = []
    chain_starts = 1 + np.arange(C0, dtype=np.int64) * L0
    jd_all = np.ascontiguousarray(dofs[chain_starts])       # [C0, 9]
    for core in range(NCORES):
        g0 = np.ascontiguousarray(
            dofs[1 + core * A0: 1 + (core + 1) * A0, :4])
        g1 = np.ascontiguousarray(
            dofs[BOFF + core * A1: BOFF + (core + 1) * A1, :4])
        jd = np.ascontiguousarray(
            jd_all[core * CH0:(core + 1) * CH0]
            .reshape(CHI, P, 9).transpose(1, 0, 2).reshape(P, CHI * 9))
        th0 = np.ascontiguousarray(
            dofs[1 + core * A0: 1 + (core + 1) * A0, 1])
        in_maps.append({"b0": g0, "b1": g1, "jd": jd, "th0": th0})
    return in_maps


def _lane_ids(id_idx, core):
    """id_idx values of this core's atoms in device lane order (p, f, t)."""
    ids0 = (id_idx[core * A0:(core + 1) * A0]
            .reshape(CHI, P, L0).transpose(1, 0, 2).ravel())
    ids1 = (id_idx[BOFF - 1 + core * A1: BOFF - 1 + (core + 1) * A1]
            .reshape(CHI, P, L1).transpose(1, 0, 2).ravel())
    return ids0, ids1


def _structure_ok(doftype, gen0_paths, gen1_paths):
    chain_starts = 1 + np.arange(C0, dtype=np.int64) * L0
    g0 = np.concatenate(
        [np.zeros((C0, 1), np.int64), chain_starts[:, None] + np.arange(L0)],
        axis=1)
    if not np.array_equal(gen0_paths, g0.astype(gen0_paths.dtype)):
        return False
    branch_roots = chain_starts + L0 // 2
    g1 = np.concatenate(
        [branch_roots[:, None],
         BOFF + (np.arange(C1, dtype=np.int64) * L1)[:, None] + np.arange(L1)],
        axis=1)
    if not np.array_equal(gen1_paths, g1.astype(gen1_paths.dtype)):
        return False
    if doftype[0] != 0:
        return False
    if not np.all(doftype[chain_starts] == 1):
        return False
    dt = doftype.copy()
    dt[chain_starts] = 2
    if not np.all(dt[1:] == 2):
        return False
    return True


def _numpy_fallback(dofs, doftype, gen0_paths, gen1_paths, id_idx):
    """Exact numpy port of the reference (slow path, safety net)."""
    def rx(a):
        c, s = np.cos(a), np.sin(a)
        o, z = np.ones_like(a), np.zeros_like(a)
        return np.stack([np.stack([o, z, z, z], -1), np.stack([z, c, -s, z], -1),
                         np.stack([z, s, c, z], -1), np.stack([z, z, z, o], -1)], -2)

    def ry(a):
        c, s = np.cos(a), np.sin(a)
        o, z = np.ones_like(a), np.zeros_like(a)
        return np.stack([np.stack([c, z, s, z], -1), np.stack([z, o, z, z], -1),
                         np.stack([-s, z, c, z], -1), np.stack([z, z, z, o], -1)], -2)

    def rz(a):
        c, s = np.cos(a), np.sin(a)
        o, z = np.ones_like(a), np.zeros_like(a)
        return np.stack([np.stack([c, -s, z, z], -1), np.stack([s, c, z, z], -1),
                         np.stack([z, z, o, z], -1), np.stack([z, z, z, o], -1)], -2)

    def trans(x, y, z):
        o, zr = np.ones_like(x), np.zeros_like(x)
        return np.stack([np.stack([o, zr, zr, x], -1), np.stack([zr, o, zr, y], -1),
                         np.stack([zr, zr, o, z], -1), np.stack([zr, zr, zr, o], -1)], -2)

    dofs = dofs.astype(np.float32)
    phi_p, theta, d, phi_c = dofs[:, 0], dofs[:, 1], dofs[:, 2], dofs[:, 3]
    z = np.zeros_like(d)
    bond = rx(phi_p) @ rz(np.pi - theta) @ trans(d, z, z) @ rx(phi_c)
    rot = lambda a, b, c: rz(c) @ ry(b) @ rx(a)
    jump = (trans(dofs[:, 0], dofs[:, 1], dofs[:, 2])
            @ rot(dofs[:, 3], dofs[:, 4], dofs[:, 5])
            @ rot(dofs[:, 6], dofs[:, 7], dofs[:, 8]))
    eye = np.broadcast_to(np.eye(4, dtype=dofs.dtype), bond.shape)
    dt = doftype[:, None, None]
    hts = np.where(dt == 1, jump, np.where(dt == 2, bond, eye)).astype(np.float32)
    for paths in (gen0_paths, gen1_paths):
        seg = hts[paths]
        out = np.empty_like(seg)
        out[:, 0] = seg[:, 0]
        for i in range(1, seg.shape[1]):
            out[:, i] = out[:, i - 1] @ seg[:, i]
        hts[paths] = out
    kincoords = hts[:, :3, 3]
    coords = np.zeros((N - 1, 3), dtype=dofs.dtype)
    coords[np.asarray(id_idx)] = kincoords[1:]
    return coords


def kernel(dofs, doftype, gen0_paths, gen1_paths, id_idx):
    dofs = np.asarray(dofs, dtype=np.float32)
    doftype = np.asarray(doftype, dtype=np.int32)
    gen0_paths = np.asarray(gen0_paths)
    gen1_paths = np.asarray(gen1_paths)
    id_idx = np.asarray(id_idx, dtype=np.int32)

    if not _structure_ok(doftype, gen0_paths, gen1_paths):
        return _numpy_fallback(dofs, doftype, gen0_paths, gen1_paths, id_idx)

    from concourse.bass_utils import run_bass_kernel_spmd

    nc = get_program()
    in_maps = _shard_inputs(dofs, doftype)
    res = run_bass_kernel_spmd(nc, in_maps, core_ids=list(range(NCORES)))
    out = np.empty((N - 1, 3), dtype=np.float32)
    for core in range(NCORES):
        ids0, ids1 = _lane_ids(id_idx, core)
        out[ids0] = res.results[core]["kin0"].reshape(-1, 3)
        out[ids1] = res.results[core]["kin1"].reshape(-1, 3)
    return out


# revision 3
# speedup vs baseline: 1.3631x; 1.3631x over previous
"""Trainium2 Bass kernel for nn_KinematicOperation — v3: fp16 planar DVE.

Same blocked-scan algorithm as v1, but the lane-parallel phases (bond fold,
level-1 scan, w, cumsum, down-transform) run in fp16 with entry-PLANAR
layouts (inner dim = lanes, unit stride, 4B aligned), which engages the
DVE's 2x_1P mode: tensor_tensor at 2 elem/cycle.  Angles stay fp32 through
the ACT sine (fp16 angle rounding would dominate the error budget); sin/cos
outputs and all downstream per-atom products are fp16.  The block-level
hierarchy (bht, levels, excl) stays fp32 packed as in v1.  ACT does the
transposing casts (trig planes, d-column, rx expansion, output repack).

Layouts per partition (generation with F lanes, T slabs):
  X[t][e][f]   e in 0..8: 3x3 entries row-major; slabs t (fp16)
  w[t][c][f]   c in 0..2 (fp16)
  trig planes nm[t*F + f] (fp16), angles atom-major fp32
  bht/lp2/spx/rx packed 12-elem HTs per lane (fp32), rx16 planar fp16
"""

import os
import sys

import numpy as np

for _p in ("/opt/trn_rl_repo", "/root/.axon_site/_ro/trn_rl_repo"):
    if os.path.isdir(_p) and _p not in sys.path:
        sys.path.insert(0, _p)

C0, L0 = 2048, 768
C1, L1 = 2048, 256
N = 1 + C0 * L0 + C1 * L1
BOFF = 1 + C0 * L0
NCORES = 8
P = 128
CHI = 2
CH0 = C0 // NCORES
CH1 = C1 // NCORES
A0 = CH0 * L0
A1 = CH1 * L1

T0, J0, S0, U0 = 12, 64, 8, 8
F0 = CHI * J0
T1, J1, S1, U1 = 8, 32, 4, 8
F1 = CHI * J1

PI = float(np.pi)

_CACHE = {}


def _build_program(repeat=1):
    from concourse import bacc, mybir, tile
    from concourse.bass import AP

    f32 = mybir.dt.float32
    f16 = mybir.dt.float16
    SIN = mybir.ActivationFunctionType.Sin
    ABS = mybir.ActivationFunctionType.Abs
    CPY = mybir.ActivationFunctionType.Copy

    nc = bacc.Bacc("TRN2", target_bir_lowering=False, debug=False)

    b0_d = nc.dram_tensor("b0", [A0, 4], f32, kind="ExternalInput")
    th0_d = nc.dram_tensor("th0", [A0], f32, kind="ExternalInput")
    b1_d = nc.dram_tensor("b1", [A1, 4], f32, kind="ExternalInput")
    jd_d = nc.dram_tensor("jd", [P, CHI * 9], f32, kind="ExternalInput")
    kin0_d = nc.dram_tensor("kin0", [P, F0 * T0 * 3], f16,
                            kind="ExternalOutput")
    kin1_d = nc.dram_tensor("kin1", [P, F1 * T1 * 3], f16,
                            kind="ExternalOutput")

    def apx(tl, off, *dims):
        t = tl[:] if not isinstance(tl, AP) else tl
        return AP(t.tensor, t.offset + off,
                  [[t.ap[0][0], P]] + [list(d) for d in dims])

    def compose_1d(E, lanes, a_off, a_step, b_off, b_step, o_off, o_step,
                   tA, tB, a_tile, b_tile, o_tile):
        for k, dst in ((0, tA), (1, tB)):
            E.tensor_mul(
                out=apx(dst, 0, (12, lanes), (4, 3), (1, 4)),
                in0=apx(a_tile, a_off + k, (a_step, lanes), (4, 3), (0, 4)),
                in1=apx(b_tile, b_off + 4 * k, (b_step, lanes), (0, 3), (1, 4)),
            )
        E.tensor_add(
            out=apx(tA, 0, (12, lanes), (1, 12)),
            in0=apx(tA, 0, (12, lanes), (1, 12)),
            in1=apx(tB, 0, (12, lanes), (1, 12)))
        E.tensor_mul(
            out=apx(tB, 0, (12, lanes), (4, 3), (1, 4)),
            in0=apx(a_tile, a_off + 2, (a_step, lanes), (4, 3), (0, 4)),
            in1=apx(b_tile, b_off + 8, (b_step, lanes), (0, 3), (1, 4)),
        )
        E.tensor_add(
            out=apx(o_tile, o_off, (o_step, lanes), (1, 12)),
            in0=apx(tA, 0, (12, lanes), (1, 12)),
            in1=apx(tB, 0, (12, lanes), (1, 12)),
        )
        E.tensor_add(
            out=apx(o_tile, o_off + 3, (o_step, lanes), (4, 3)),
            in0=apx(o_tile, o_off + 3, (o_step, lanes), (4, 3)),
            in1=apx(a_tile, a_off + 3, (a_step, lanes), (4, 3)),
        )

    def excl_blocks(E, SC, CS, U, LPS, spx, lp2, rx, tA, tB):
        SC.copy(out=apx(rx, 0, (U * 12, CS), (1, 12)),
                in_=apx(spx, 0, (12, CS), (1, 12)))
        UM = U - 1
        for i in range(3):
            for k, dst in ((0, tA), (1, tB)):
                E.tensor_mul(
                    out=apx(dst, 4 * i, (96, CS), (12, UM), (1, 4)),
                    in0=apx(spx, 4 * i + k, (12, CS), (0, UM), (0, 4)),
                    in1=apx(lp2, 12 + 4 * k, (LPS, CS), (12, UM), (1, 4)))
            E.tensor_add(
                out=apx(tA, 4 * i, (96, CS), (12, UM), (1, 4)),
                in0=apx(tA, 4 * i, (96, CS), (12, UM), (1, 4)),
                in1=apx(tB, 4 * i, (96, CS), (12, UM), (1, 4)))
            E.tensor_mul(
                out=apx(tB, 4 * i, (96, CS), (12, UM), (1, 4)),
                in0=apx(spx, 4 * i + 2, (12, CS), (0, UM), (0, 4)),
                in1=apx(lp2, 12 + 8, (LPS, CS), (12, UM), (1, 4)))
            E.tensor_add(
                out=apx(rx, 12 + 4 * i, (96, CS), (12, UM), (1, 4)),
                in0=apx(tA, 4 * i, (96, CS), (12, UM), (1, 4)),
                in1=apx(tB, 4 * i, (96, CS), (12, UM), (1, 4)))
        E.tensor_add(
            out=apx(rx, 12 + 3, (96, CS), (12, UM), (4, 3)),
            in0=apx(rx, 12 + 3, (96, CS), (12, UM), (4, 3)),
            in1=apx(spx, 3, (12, CS), (0, UM), (4, 3)))

    with tile.TileContext(nc) as tc:
      for _rep in range(repeat):
        with tc.tile_pool(name="main", bufs=1) as mp:
            X0 = mp.tile([P, T0 * F0 * 9], f16)
            w0 = mp.tile([P, T0 * F0 * 3], f16)
            tW1_0 = mp.tile([P, T0 * F0], f16)
            tW2_0 = mp.tile([P, T0 * F0], f16)
            tA0 = mp.tile([P, 4 * F0], f16)
            tB0 = mp.tile([P, 6 * F0], f16)
            tC0 = mp.tile([P, 6 * F0], f16)
            tg0 = {nm: mp.tile([P, T0 * F0], f16, name=f"tg0_{nm}")
                   for nm in ("sa", "ca", "st", "ct")}
            apl0 = mp.tile([P, CHI * L0], f32)
            wsc0 = mp.tile([P, CHI * L0], f32)
            aw0 = mp.tile([P, CHI * L0], f32)
            d16_0 = mp.tile([P, T0 * F0], f16)
            tg1 = {nm: mp.tile([P, T1 * F1], f16, name=f"tg1_{nm}")
                   for nm in ("sa", "ca", "st", "ct")}
            apl1 = mp.tile([P, CHI * L1], f32)
            wsc1 = mp.tile([P, CHI * L1], f32)
            aw1 = mp.tile([P, CHI * L1], f32)
            d16_1 = mp.tile([P, T1 * F1], f16)
            bht0 = mp.tile([P, F0 * 12], f32)
            rx0 = mp.tile([P, F0 * 12], f32)
            rx16_0 = mp.tile([P, F0 * 12], f16)
            lp2_0 = mp.tile([P, CHI * S0 * (U0 + 1) * 12], f32)
            spx0 = mp.tile([P, CHI * S0 * 12], f32)
            lp2_1 = mp.tile([P, CHI * S1 * (U1 + 1) * 12], f32)
            spx1 = mp.tile([P, CHI * S1 * 12], f32)
            tAh = mp.tile([P, 96 * CHI * S0], f32)
            tBh = mp.tile([P, 96 * CHI * S0], f32)
            a32t = mp.tile([P, CHI * 12], f32)
            rbr = mp.tile([P, CHI * 12], f32)
            jd = mp.tile([P, CHI * 9], f32)
            jang = mp.tile([P, CHI * 2 * 3], f32)
            jsin = mp.tile([P, CHI * 2 * 3], f32)
            jcos = mp.tile([P, CHI * 2 * 3], f32)
            re_ = mp.tile([P, CHI * 2 * 9], f32)
            rj = mp.tile([P, CHI * 9], f32)
            jtmp = mp.tile([P, CHI * 2 * 9], f32)
            halfpi = mp.tile([P, 1], f32)

            V = nc.vector
            SC = nc.scalar

            nc.sync.dma_start(out=jd[:], in_=jd_d[:])
            V.memset(halfpi[:], PI / 2)

            # prefill identities (fp32 hierarchy tiles)
            V.memset(lp2_0[:], 0.0)
            V.memset(apx(lp2_0, 0, ((U0 + 1) * 12, CHI * S0), (5, 3)), 1.0)
            V.memset(spx0[:], 0.0)
            V.memset(apx(spx0, 0, (S0 * 12, CHI), (5, 3)), 1.0)
            V.memset(lp2_1[:], 0.0)
            V.memset(apx(lp2_1, 0, ((U1 + 1) * 12, CHI * S1), (5, 3)), 1.0)
            V.memset(apx(X0, 2 * F0, (1, F0)), 0.0)  # slab0 e=2 plane

            # ---- JUMP HT build (fp32, tiny) ----
            V.tensor_copy(out=jang[:], in_=apx(jd, 3, (9, CHI), (3, 2),
                                               (1, 3)))
            V.add_range_wrap(out=jsin[:], in_=jang[:], shift=0.0,
                             bound=PI, period=2 * PI)
            SC.activation(out=jsin[:], in_=jsin[:], func=SIN)
            V.add_range_wrap(out=jcos[:], in_=jang[:], shift=PI / 2,
                             bound=PI, period=2 * PI)
            SC.activation(out=jcos[:], in_=jcos[:], func=SIN)

            CR = CHI * 2

            def sc_(tl, ang):
                return apx(tl, ang, (3, CR))

            def re(e):
                return apx(re_, e, (9, CR))

            def jt1(e):
                return apx(jtmp, e, (9, CR))

            sa_ = lambda: sc_(jsin, 0)
            sb = lambda: sc_(jsin, 1)
            s_c = lambda: sc_(jsin, 2)
            ca_ = lambda: sc_(jcos, 0)
            cb = lambda: sc_(jcos, 1)
            c_c = lambda: sc_(jcos, 2)
            V.tensor_mul(out=re(0), in0=c_c(), in1=cb())
            V.tensor_mul(out=jt1(0), in0=sb(), in1=sa_())
            V.tensor_mul(out=jt1(1), in0=sb(), in1=ca_())
            V.tensor_mul(out=jt1(2), in0=c_c(), in1=jt1(0))
            V.tensor_mul(out=jt1(3), in0=s_c(), in1=ca_())
            V.tensor_sub(out=re(1), in0=jt1(2), in1=jt1(3))
            V.tensor_mul(out=jt1(2), in0=c_c(), in1=jt1(1))
            V.tensor_mul(out=jt1(3), in0=s_c(), in1=sa_())
            V.tensor_add(out=re(2), in0=jt1(2), in1=jt1(3))
            V.tensor_mul(out=re(3), in0=s_c(), in1=cb())
            V.tensor_mul(out=jt1(2), in0=s_c(), in1=jt1(0))
            V.tensor_mul(out=jt1(3), in0=c_c(), in1=ca_())
            V.tensor_add(out=re(4), in0=jt1(2), in1=jt1(3))
            V.tensor_mul(out=jt1(2), in0=s_c(), in1=jt1(1))
            V.tensor_mul(out=jt1(3), in0=c_c(), in1=sa_())
            V.tensor_sub(out=re(5), in0=jt1(2), in1=jt1(3))
            V.tensor_scalar_mul(out=re(6), in0=sb(), scalar1=-1.0)
            V.tensor_mul(out=re(7), in0=cb(), in1=sa_())
            V.tensor_mul(out=re(8), in0=cb(), in1=ca_())
            V.tensor_mul(
                out=apx(rj, 0, (9, CHI), (3, 3), (1, 3)),
                in0=apx(re_, 0, (18, CHI), (3, 3), (0, 3)),
                in1=apx(re_, 9, (18, CHI), (0, 3), (1, 3)))
            V.tensor_mul(
                out=apx(jtmp, 0, (9, CHI), (3, 3), (1, 3)),
                in0=apx(re_, 1, (18, CHI), (3, 3), (0, 3)),
                in1=apx(re_, 12, (18, CHI), (0, 3), (1, 3)))
            V.tensor_add(out=rj[:, : CHI * 9], in0=rj[:, : CHI * 9],
                         in1=jtmp[:, : CHI * 9])
            V.tensor_mul(
                out=apx(jtmp, 0, (9, CHI), (3, 3), (1, 3)),
                in0=apx(re_, 2, (18, CHI), (3, 3), (0, 3)),
                in1=apx(re_, 15, (18, CHI), (0, 3), (1, 3)))
            V.tensor_add(out=rj[:, : CHI * 9], in0=rj[:, : CHI * 9],
                         in1=jtmp[:, : CHI * 9])

            # ======== trig (both gens) over the dof pool ========
            def emit_trig(dof, L, F, T, tg, apl, wsc, aw, afix, theta_src):
                # theta: wrap (V, atom-major) -> sin/abs/sin (ACT) into
                # t-major fp16 planes
                V.add_range_wrap(out=wsc[:], in_=theta_src, shift=0.0,
                                 bound=PI, period=2 * PI)
                SC.activation(out=apx(tg["st"], 0, (1, F), (F, T)),
                              in_=apx(wsc, 0, (T, F), (1, T)), func=SIN)
                SC.activation(out=aw[:], in_=wsc[:], func=ABS)
                SC.activation(out=apx(tg["ct"], 0, (1, F), (F, T)),
                              in_=apx(aw, 0, (T, F), (1, T)), func=SIN,
                              scale=-1.0, bias=halfpi[:])
                V.tensor_add(out=apx(apl, 1, (L, CHI), (1, L - 1)),
                             in0=apx(dof, 4, (L * 4, CHI), (4, L - 1)),
                             in1=apx(dof, 3, (L * 4, CHI), (4, L - 1)))
                afix(apl)
                V.add_range_wrap(out=wsc[:], in_=apl[:], shift=0.0,
                                 bound=PI, period=2 * PI)
                SC.activation(out=apx(tg["sa"], 0, (1, F), (F, T)),
                              in_=apx(wsc, 0, (T, F), (1, T)), func=SIN)
                SC.activation(out=aw[:], in_=wsc[:], func=ABS)
                SC.activation(out=apx(tg["ca"], 0, (1, F), (F, T)),
                              in_=apx(aw, 0, (T, F), (1, T)), func=SIN,
                              scale=-1.0, bias=halfpi[:])

            def emit_fold(X, tg, F, T):
                def tp(nm):
                    return apx(tg[nm], 0, (F, T), (1, F))

                def xo(e):
                    return apx(X, e * F, (9 * F, T), (1, F))

                SC.activation(out=xo(0), in_=tp("ct"), func=CPY, scale=-1.0)
                SC.activation(out=xo(1), in_=tp("st"), func=CPY, scale=-1.0)
                SC.activation(out=xo(5), in_=tp("sa"), func=CPY, scale=-1.0)
                SC.activation(out=xo(8), in_=tp("ca"), func=CPY)
                V.tensor_mul(out=xo(3), in0=tp("ca"), in1=tp("st"))
                V.tensor_mul(out=xo(4), in0=tp("ca"), in1=xo(0))
                V.tensor_mul(out=xo(6), in0=tp("sa"), in1=tp("st"))
                V.tensor_mul(out=xo(7), in0=tp("sa"), in1=xo(0))

            with tc.tile_pool(name="pdof", bufs=1) as pd:
                dof0 = pd.tile([P, CHI * L0 * 4], f32)
                dof1 = pd.tile([P, CHI * L1 * 4], f32)
                th0 = pd.tile([P, CHI * L0], f32)

                src = AP(th0_d, 0, [[L0, P], [P * L0, CHI], [1, L0]])
                dst = AP(th0[:].tensor, th0[:].offset,
                         [[th0[:].ap[0][0], P], [L0, CHI], [1, L0]])
                nc.sync.dma_start(out=dst, in_=src)
                src = AP(b0_d, 0, [[L0 * 4, P], [P * L0 * 4, CHI],
                                   [1, L0 * 4]])
                dst = AP(dof0[:].tensor, dof0[:].offset,
                         [[dof0[:].ap[0][0], P], [L0 * 4, CHI], [1, L0 * 4]])
                nc.sync.dma_start(out=dst, in_=src)
                src = AP(b1_d, 0, [[L1 * 4, P], [P * L1 * 4, CHI],
                                   [1, L1 * 4]])
                dst = AP(dof1[:].tensor, dof1[:].offset,
                         [[dof1[:].ap[0][0], P], [L1 * 4, CHI], [1, L1 * 4]])
                nc.sync.dma_start(out=dst, in_=src)

                def afix0(apl):
                    V.tensor_copy(out=apx(apl, 1, (L0, CHI)),
                                  in_=apx(dof0, 4, (L0 * 4, CHI)))

                emit_trig(dof0, L0, F0, T0, tg0, apl0, wsc0, aw0, afix0,
                          th0[:])
                # d16: dof col2 -> planar fp16
                SC.activation(out=apx(d16_0, 0, (1, F0), (F0, T0)),
                              in_=apx(dof0, 2, (T0 * 4, F0), (4, T0)),
                              func=CPY)
                emit_fold(X0, tg0, F0, T0)
                # jump 3x3 into X0 slab0 lanes chi*J0
                V.tensor_copy(out=apx(X0, 0, (F0, 9), (J0, CHI)),
                              in_=apx(rj, 0, (1, 9), (9, CHI)))

                def afix1(apl):
                    V.tensor_add(out=apx(apl, 0, (L1, CHI)),
                                 in0=apx(dof1, 0, (L1 * 4, CHI)),
                                 in1=apx(dof0, 384 * 4 + 3, (L0 * 4, CHI)))

                emit_trig(dof1, L1, F1, T1, tg1, apl1, wsc1, aw1, afix1,
                          apx(dof1, 1, (L1 * 4, CHI), (4, L1)))
                SC.activation(out=apx(d16_1, 0, (1, F1), (F1, T1)),
                              in_=apx(dof1, 2, (T1 * 4, F1), (4, T1)),
                              func=CPY)

            # ======== scans + rest ========
            with tc.tile_pool(name="px1", bufs=1) as px:
                X1 = px.tile([P, T1 * F1 * 9], f16)
                w1 = px.tile([P, T1 * F1 * 3], f16)
                tW1_1 = px.tile([P, T1 * F1], f16)
                tW2_1 = px.tile([P, T1 * F1], f16)
                tA1 = px.tile([P, 4 * F1], f16)
                tB1 = px.tile([P, 6 * F1], f16)
                tC1 = px.tile([P, 6 * F1], f16)
                bht1 = px.tile([P, F1 * 12], f32)
                rx1 = px.tile([P, F1 * 12], f32)
                rx16_1 = px.tile([P, F1 * 12], f16)
                tA1h = px.tile([P, 96 * CHI * S1], f32)
                tB1h = px.tile([P, 96 * CHI * S1], f32)

                V.memset(apx(X1, 2 * F1, (1, F1)), 0.0)
                emit_fold(X1, tg1, F1, T1)

                def scan_i(X, tA, tB, tC, F, t, i):
                    pb = (t - 1) * 9 * F
                    cb = t * 9 * F
                    if i == 0:
                        V.tensor_mul(
                            out=apx(tA, 0, (2 * F, 2), (F, 2), (1, F)),
                            in0=apx(X, pb, (3 * F, 2), (0, 2), (1, F)),
                            in1=apx(X, cb, (0, 2), (F, 2), (1, F)))
                    elif i == 1:
                        V.tensor_mul(
                            out=apx(tB, 0, (3 * F, 2), (F, 3), (1, F)),
                            in0=apx(X, pb + F, (3 * F, 2), (0, 3), (1, F)),
                            in1=apx(X, cb + 3 * F, (0, 2), (F, 3), (1, F)))
                    elif i == 2:
                        V.tensor_mul(
                            out=apx(tC, 0, (3 * F, 2), (F, 3), (1, F)),
                            in0=apx(X, pb + 2 * F, (3 * F, 2), (0, 3),
                                    (1, F)),
                            in1=apx(X, cb + 6 * F, (0, 2), (F, 3), (1, F)))
                    elif i == 3:
                        V.tensor_add(
                            out=apx(tA, 0, (2 * F, 2), (F, 2), (1, F)),
                            in0=apx(tA, 0, (2 * F, 2), (F, 2), (1, F)),
                            in1=apx(tB, 0, (3 * F, 2), (F, 2), (1, F)))
                    elif i == 4:
                        V.tensor_add(
                            out=apx(X, cb, (3 * F, 2), (F, 2), (1, F)),
                            in0=apx(tA, 0, (2 * F, 2), (F, 2), (1, F)),
                            in1=apx(tC, 0, (3 * F, 2), (F, 2), (1, F)))
                    else:
                        V.tensor_add(
                            out=apx(X, cb + 2 * F, (3 * F, 2), (1, F)),
                            in0=apx(tB, 2 * F, (3 * F, 2), (1, F)),
                            in1=apx(tC, 2 * F, (3 * F, 2), (1, F)))

                for t in range(1, T0):
                    for i in range(6):
                        scan_i(X0, tA0, tB0, tC0, F0, t, i)
                        if t < T1:
                            scan_i(X1, tA1, tB1, tC1, F1, t, i)

                def emit_w(X, w, tW1, tW2, d16, F, T):
                    V.tensor_mul(out=apx(tW1, 0, (F, T), (1, F)),
                                 in0=apx(X, F, (9 * F, T), (1, F)),
                                 in1=apx(X, 5 * F, (9 * F, T), (1, F)))
                    V.tensor_mul(out=apx(tW2, 0, (F, T), (1, F)),
                                 in0=apx(X, 2 * F, (9 * F, T), (1, F)),
                                 in1=apx(X, 4 * F, (9 * F, T), (1, F)))
                    V.tensor_sub(out=apx(tW1, 0, (F, T), (1, F)),
                                 in0=apx(tW1, 0, (F, T), (1, F)),
                                 in1=apx(tW2, 0, (F, T), (1, F)))
                    V.tensor_mul(out=apx(w, 2 * F, (3 * F, T), (1, F)),
                                 in0=apx(tW1, 0, (F, T), (1, F)),
                                 in1=apx(d16, 0, (F, T), (1, F)))
                    V.tensor_mul(out=apx(w, 0, (3 * F, T), (F, 2), (1, F)),
                                 in0=apx(X, 0, (9 * F, T), (3 * F, 2),
                                         (1, F)),
                                 in1=apx(d16, 0, (F, T), (0, 2), (1, F)))

                emit_w(X0, w0, tW1_0, tW2_0, d16_0, F0, T0)
                emit_w(X1, w1, tW1_1, tW2_1, d16_1, F1, T1)

                # jump translation into w0 slab0 lanes chi*J0
                V.tensor_copy(out=apx(w0, 0, (F0, 3), (J0, CHI)),
                              in_=apx(jd, 0, (1, 3), (9, CHI)))

                # a32: in-block HT of branch root (lane j=32 per chi, t=0)
                V.tensor_copy(out=apx(a32t, 0, (12, CHI), (4, 2), (1, 3)),
                              in_=apx(X0, 32, (J0, CHI), (3 * F0, 2),
                                      (F0, 3)))
                SC.copy(out=apx(a32t, 8, (12, CHI)),
                        in_=apx(tW1_0, 32, (J0, CHI)))
                for dsti, (e1, e2), (e3, e4) in ((9, (2, 3), (0, 5)),
                                                 (10, (0, 4), (1, 3))):
                    V.tensor_mul(out=apx(tAh, 0, (1, CHI)),
                                 in0=apx(X0, 32 + e1 * F0, (J0, CHI)),
                                 in1=apx(X0, 32 + e2 * F0, (J0, CHI)))
                    V.tensor_mul(out=apx(tBh, 0, (1, CHI)),
                                 in0=apx(X0, 32 + e3 * F0, (J0, CHI)),
                                 in1=apx(X0, 32 + e4 * F0, (J0, CHI)))
                    V.tensor_sub(out=apx(a32t, dsti, (12, CHI)),
                                 in0=apx(tAh, 0, (1, CHI)),
                                 in1=apx(tBh, 0, (1, CHI)))

                # cumsums (slab-contiguous fp16)
                for t in range(1, T0):
                    V.tensor_add(
                        out=apx(w0, t * 3 * F0, (1, 3 * F0)),
                        in0=apx(w0, t * 3 * F0, (1, 3 * F0)),
                        in1=apx(w0, (t - 1) * 3 * F0, (1, 3 * F0)))
                    if t < T1:
                        V.tensor_add(
                            out=apx(w1, t * 3 * F1, (1, 3 * F1)),
                            in0=apx(w1, t * 3 * F1, (1, 3 * F1)),
                            in1=apx(w1, (t - 1) * 3 * F1, (1, 3 * F1)))

                # a32 translation (slab 0 of cumsum = w slab 0)
                V.tensor_copy(out=apx(a32t, 3, (12, CHI), (4, 3)),
                              in_=apx(w0, 32, (J0, CHI), (F0, 3)))

                # block-total HTs -> fp32 packed bht
                def emit_bht(X, w, tW1, bht, F, T):
                    base = (T - 1) * 9 * F
                    SC.copy(out=apx(bht, 0, (12, F), (4, 2), (1, 3)),
                            in_=apx(X, base, (1, F), (3 * F, 2), (F, 3)))
                    SC.copy(out=apx(bht, 8, (12, F)),
                            in_=apx(tW1, (T - 1) * F, (1, F)))
                    for dsti, (e1, e2), (e3, e4) in ((9, (2, 3), (0, 5)),
                                                     (10, (0, 4), (1, 3))):
                        V.tensor_mul(out=apx(tAh, 0, (1, F)),
                                     in0=apx(X, base + e1 * F, (1, F)),
                                     in1=apx(X, base + e2 * F, (1, F)))
                        V.tensor_mul(out=apx(tBh, 0, (1, F)),
                                     in0=apx(X, base + e3 * F, (1, F)),
                                     in1=apx(X, base + e4 * F, (1, F)))
                        V.tensor_sub(out=apx(bht, dsti, (12, F)),
                                     in0=apx(tAh, 0, (1, F)),
                                     in1=apx(tBh, 0, (1, F)))
                    SC.copy(out=apx(bht, 3, (12, F), (4, 3)),
                            in_=apx(w, (T - 1) * 3 * F, (1, F), (F, 3)))

                emit_bht(X0, w0, tW1_0, bht0, F0, T0)
                emit_bht(X1, w1, tW1_1, bht1, F1, T1)

                # ---- hierarchy (fp32, as v1) ----
                LPS0 = (U0 + 1) * 12
                LPS1 = (U1 + 1) * 12
                SC.copy(out=apx(lp2_0, 12, (LPS0, CHI * S0), (1, 12)),
                        in_=apx(bht0, 0, (U0 * 12, CHI * S0), (1, 12)))
                SC.copy(out=apx(lp2_1, 12, (LPS1, CHI * S1), (1, 12)),
                        in_=apx(bht1, 0, (U1 * 12, CHI * S1), (1, 12)))
                for u in range(1, U0):
                    compose_1d(V, CHI * S0,
                               a_off=u * 12, a_step=LPS0,
                               b_off=u * 12, b_step=U0 * 12,
                               o_off=(u + 1) * 12, o_step=LPS0,
                               tA=tAh, tB=tBh,
                               a_tile=lp2_0, b_tile=bht0, o_tile=lp2_0)
                    if u < U1:
                        compose_1d(V, CHI * S1,
                                   a_off=u * 12, a_step=LPS1,
                                   b_off=u * 12, b_step=U1 * 12,
                                   o_off=(u + 1) * 12, o_step=LPS1,
                                   tA=tA1h, tB=tB1h,
                                   a_tile=lp2_1, b_tile=bht1, o_tile=lp2_1)
                for sidx in range(1, S0):
                    compose_1d(V, CHI,
                               a_off=(sidx - 1) * 12, a_step=S0 * 12,
                               b_off=(sidx - 1) * LPS0 + U0 * 12,
                               b_step=S0 * LPS0,
                               o_off=sidx * 12, o_step=S0 * 12,
                               tA=tAh, tB=tBh,
                               a_tile=spx0, b_tile=lp2_0, o_tile=spx0)
                excl_blocks(V, SC, CHI * S0, U0, LPS0, spx0, lp2_0, rx0,
                            tAh, tBh)
                compose_1d(V, CHI,
                           a_off=32 * 12, a_step=J0 * 12,
                           b_off=0, b_step=12,
                           o_off=0, o_step=12,
                           tA=tAh, tB=tBh,
                           a_tile=rx0, b_tile=a32t, o_tile=rbr)
                SC.copy(out=apx(spx1, 0, (S1 * 12, CHI), (1, 12)),
                        in_=apx(rbr, 0, (12, CHI), (1, 12)))
                # rx -> planar fp16 for the down transform
                SC.copy(out=apx(rx16_0, 0, (1, F0), (F0, 12)),
                        in_=apx(rx0, 0, (12, F0), (1, 12)))

                def down_i(w, rx16, X, tmpoff, F, T, i):
                    xyz = apx(X, 0, (3 * F, T), (F, 3), (1, F))
                    tmp = apx(X, tmpoff, (3 * F, T), (F, 3), (1, F))

                    def rxk(k):
                        return apx(rx16, k * F, (0, T), (4 * F, 3), (1, F))

                    def wk(k):
                        return apx(w, k * F, (3 * F, T), (0, 3), (1, F))

                    if i == 0:
                        V.tensor_mul(out=xyz, in0=rxk(0), in1=wk(0))
                    elif i == 1:
                        V.tensor_mul(out=tmp, in0=rxk(1), in1=wk(1))
                    elif i == 2:
                        V.tensor_add(out=xyz, in0=xyz, in1=tmp)
                    elif i == 3:
                        V.tensor_mul(out=tmp, in0=rxk(2), in1=wk(2))
                    elif i == 4:
                        V.tensor_add(out=xyz, in0=xyz, in1=tmp)
                    else:
                        V.tensor_add(out=xyz, in0=xyz, in1=rxk(3))

                # gen1 level-3 + excl interleaved with down-g0
                for sidx in range(1, S1):
                    compose_1d(V, CHI,
                               a_off=(sidx - 1) * 12, a_step=S1 * 12,
                               b_off=(sidx - 1) * LPS1 + U1 * 12,
                               b_step=S1 * LPS1,
                               o_off=sidx * 12, o_step=S1 * 12,
                               tA=tA1h, tB=tB1h,
                               a_tile=spx1, b_tile=lp2_1, o_tile=spx1)
                    down_i(w0, rx16_0, X0, 3 * F0 * T0, F0, T0, sidx - 1)
                excl_blocks(V, SC, CHI * S1, U1, LPS1, spx1, lp2_1, rx1,
                            tA1h, tB1h)
                SC.copy(out=apx(rx16_1, 0, (1, F1), (F1, 12)),
                        in_=apx(rx1, 0, (12, F1), (1, 12)))
                for i in range(3, 6):
                    down_i(w0, rx16_0, X0, 3 * F0 * T0, F0, T0, i)
                # repack xyz planar -> packed (f, t, c) fp16 into w0; DMA out
                SC.copy(out=apx(w0, 0, (T0 * 3, F0), (3, T0), (1, 3)),
                        in_=apx(X0, 0, (1, F0), (3 * F0, T0), (F0, 3)))
                nc.sync.dma_start(
                    out=AP(kin0_d, 0, [[F0 * T0 * 3, P], [1, F0 * T0 * 3]]),
                    in_=apx(w0, 0, (1, F0 * T0 * 3)))
                for i in range(6):
                    down_i(w1, rx16_1, X1, 3 * F1 * T1, F1, T1, i)
                SC.copy(out=apx(w1, 0, (T1 * 3, F1), (3, T1), (1, 3)),
                        in_=apx(X1, 0, (1, F1), (3 * F1, T1), (F1, 3)))
                nc.sync.dma_start(
                    out=AP(kin1_d, 0, [[F1 * T1 * 3, P], [1, F1 * T1 * 3]]),
                    in_=apx(w1, 0, (1, F1 * T1 * 3)))

    nc.compile()
    return nc


def get_program(repeat=1):
    key = ("nc", repeat)
    if key not in _CACHE:
        _CACHE[key] = _build_program(repeat)
    return _CACHE[key]


# ------------------------------------------------------------------- host
def _shard_inputs(dofs, doftype):
    """Build the 8 per-core input maps (lane order (p, chi, j, t))."""
    in_maps = []
    chain_starts = 1 + np.arange(C0, dtype=np.int64) * L0
    jd_all = np.ascontiguousarray(dofs[chain_starts])       # [C0, 9]
    for core in range(NCORES):
        g0 = np.ascontiguousarray(
            dofs[1 + core * A0: 1 + (core + 1) * A0, :4])
        g1 = np.ascontiguousarray(
            dofs[BOFF + core * A1: BOFF + (core + 1) * A1, :4])
        jd = np.ascontiguousarray(
            jd_all[core * CH0:(core + 1) * CH0]
            .reshape(CHI, P, 9).transpose(1, 0, 2).reshape(P, CHI * 9))
        th0 = np.ascontiguousarray(
            dofs[1 + core * A0: 1 + (core + 1) * A0, 1])
        in_maps.append({"b0": g0, "b1": g1, "jd": jd, "th0": th0})
    return in_maps


def _lane_ids(id_idx, core):
    """id_idx values of this core's atoms in device lane order (p, f, t)."""
    ids0 = (id_idx[core * A0:(core + 1) * A0]
            .reshape(CHI, P, L0).transpose(1, 0, 2).ravel())
    ids1 = (id_idx[BOFF - 1 + core * A1: BOFF - 1 + (core + 1) * A1]
            .reshape(CHI, P, L1).transpose(1, 0, 2).ravel())
    return ids0, ids1


def _structure_ok(doftype, gen0_paths, gen1_paths):
    chain_starts = 1 + np.arange(C0, dtype=np.int64) * L0
    g0 = np.concatenate(
        [np.zeros((C0, 1), np.int64), chain_starts[:, None] + np.arange(L0)],
        axis=1)
    if not np.array_equal(gen0_paths, g0.astype(gen0_paths.dtype)):
        return False
    branch_roots = chain_starts + L0 // 2
    g1 = np.concatenate(
        [branch_roots[:, None],
         BOFF + (np.arange(C1, dtype=np.int64) * L1)[:, None] + np.arange(L1)],
        axis=1)
    if not np.array_equal(gen1_paths, g1.astype(gen1_paths.dtype)):
        return False
    if doftype[0] != 0:
        return False
    if not np.all(doftype[chain_starts] == 1):
        return False
    dt = doftype.copy()
    dt[chain_starts] = 2
    if not np.all(dt[1:] == 2):
        return False
    return True


def _numpy_fallback(dofs, doftype, gen0_paths, gen1_paths, id_idx):
    """Exact numpy port of the reference (slow path, safety net)."""
    def rx(a):
        c, s = np.cos(a), np.sin(a)
        o, z = np.ones_like(a), np.zeros_like(a)
        return np.stack([np.stack([o, z, z, z], -1), np.stack([z, c, -s, z], -1),
                         np.stack([z, s, c, z], -1), np.stack([z, z, z, o], -1)], -2)

    def ry(a):
        c, s = np.cos(a), np.sin(a)
        o, z = np.ones_like(a), np.zeros_like(a)
        return np.stack([np.stack([c, z, s, z], -1), np.stack([z, o, z, z], -1),
                         np.stack([-s, z, c, z], -1), np.stack([z, z, z, o], -1)], -2)

    def rz(a):
        c, s = np.cos(a), np.sin(a)
        o, z = np.ones_like(a), np.zeros_like(a)
        return np.stack([np.stack([c, -s, z, z], -1), np.stack([s, c, z, z], -1),
                         np.stack([z, z, o, z], -1), np.stack([z, z, z, o], -1)], -2)

    def trans(x, y, z):
        o, zr = np.ones_like(x), np.zeros_like(x)
        return np.stack([np.stack([o, zr, zr, x], -1), np.stack([zr, o, zr, y], -1),
                         np.stack([zr, zr, o, z], -1), np.stack([zr, zr, zr, o], -1)], -2)

    dofs = dofs.astype(np.float32)
    phi_p, theta, d, phi_c = dofs[:, 0], dofs[:, 1], dofs[:, 2], dofs[:, 3]
    z = np.zeros_like(d)
    bond = rx(phi_p) @ rz(np.pi - theta) @ trans(d, z, z) @ rx(phi_c)
    rot = lambda a, b, c: rz(c) @ ry(b) @ rx(a)
    jump = (trans(dofs[:, 0], dofs[:, 1], dofs[:, 2])
            @ rot(dofs[:, 3], dofs[:, 4], dofs[:, 5])
            @ rot(dofs[:, 6], dofs[:, 7], dofs[:, 8]))
    eye = np.broadcast_to(np.eye(4, dtype=dofs.dtype), bond.shape)
    dt = doftype[:, None, None]
    hts = np.where(dt == 1, jump, np.where(dt == 2, bond, eye)).astype(np.float32)
    for paths in (gen0_paths, gen1_paths):
        seg = hts[paths]
        out = np.empty_like(seg)
        out[:, 0] = seg[:, 0]
        for i in range(1, seg.shape[1]):
            out[:, i] = out[:, i - 1] @ seg[:, i]
        hts[paths] = out
    kincoords = hts[:, :3, 3]
    coords = np.zeros((N - 1, 3), dtype=dofs.dtype)
    coords[np.asarray(id_idx)] = kincoords[1:]
    return coords


def kernel(dofs, doftype, gen0_paths, gen1_paths, id_idx):
    dofs = np.asarray(dofs, dtype=np.float32)
    doftype = np.asarray(doftype, dtype=np.int32)
    gen0_paths = np.asarray(gen0_paths)
    gen1_paths = np.asarray(gen1_paths)
    id_idx = np.asarray(id_idx, dtype=np.int32)

    if not _structure_ok(doftype, gen0_paths, gen1_paths):
        return _numpy_fallback(dofs, doftype, gen0_paths, gen1_paths, id_idx)

    from concourse.bass_utils import run_bass_kernel_spmd

    nc = get_program()
    in_maps = _shard_inputs(dofs, doftype)
    res = run_bass_kernel_spmd(nc, in_maps, core_ids=list(range(NCORES)))
    out = np.empty((N - 1, 3), dtype=np.float32)
    for core in range(NCORES):
        ids0, ids1 = _lane_ids(id_idx, core)
        out[ids0] = res.results[core]["kin0"].astype(np.float32).reshape(-1, 3)
        out[ids1] = res.results[core]["kin1"].astype(np.float32).reshape(-1, 3)
    return out


# revision 5
# speedup vs baseline: 1.4428x; 1.0585x over previous
"""Trainium2 Bass kernel for nn_KinematicOperation — v3: fp16 planar DVE.

Same blocked-scan algorithm as v1, but the lane-parallel phases (bond fold,
level-1 scan, w, cumsum, down-transform) run in fp16 with entry-PLANAR
layouts (inner dim = lanes, unit stride, 4B aligned), which engages the
DVE's 2x_1P mode: tensor_tensor at 2 elem/cycle.  Angles stay fp32 through
the ACT sine (fp16 angle rounding would dominate the error budget); sin/cos
outputs and all downstream per-atom products are fp16.  The block-level
hierarchy (bht, levels, excl) stays fp32 packed as in v1.  ACT does the
transposing casts (trig planes, d-column, rx expansion, output repack).

Layouts per partition (generation with F lanes, T slabs):
  X[t][e][f]   e in 0..8: 3x3 entries row-major; slabs t (fp16)
  w[t][c][f]   c in 0..2 (fp16)
  trig planes nm[t*F + f] (fp16), angles atom-major fp32
  bht/lp2/spx/rx packed 12-elem HTs per lane (fp32), rx16 planar fp16
"""

import os
import sys

import numpy as np

for _p in ("/opt/trn_rl_repo", "/root/.axon_site/_ro/trn_rl_repo"):
    if os.path.isdir(_p) and _p not in sys.path:
        sys.path.insert(0, _p)

C0, L0 = 2048, 768
C1, L1 = 2048, 256
N = 1 + C0 * L0 + C1 * L1
BOFF = 1 + C0 * L0
NCORES = 8
P = 128
CHI = 2
CH0 = C0 // NCORES
CH1 = C1 // NCORES
A0 = CH0 * L0
A1 = CH1 * L1

T0, J0, S0, U0 = 12, 64, 8, 8
F0 = CHI * J0
T1, J1, S1, U1 = 8, 32, 4, 8
F1 = CHI * J1

PI = float(np.pi)

_CACHE = {}


def _build_program(repeat=1):
    from concourse import bacc, mybir, tile
    from concourse.bass import AP

    f32 = mybir.dt.float32
    f16 = mybir.dt.float16
    SIN = mybir.ActivationFunctionType.Sin
    ABS = mybir.ActivationFunctionType.Abs
    CPY = mybir.ActivationFunctionType.Copy

    nc = bacc.Bacc("TRN2", target_bir_lowering=False, debug=False)

    th0_d = nc.dram_tensor("th0", [P, CHI * L0], f32, kind="ExternalInput")
    al0_d = nc.dram_tensor("al0", [P, CHI * L0], f32, kind="ExternalInput")
    dt0_d = nc.dram_tensor("dt0", [P, CHI * L0], f32, kind="ExternalInput")
    th1_d = nc.dram_tensor("th1", [P, CHI * L1], f32, kind="ExternalInput")
    al1_d = nc.dram_tensor("al1", [P, CHI * L1], f32, kind="ExternalInput")
    dt1_d = nc.dram_tensor("dt1", [P, CHI * L1], f32, kind="ExternalInput")
    jd_d = nc.dram_tensor("jd", [P, CHI * 9], f32, kind="ExternalInput")
    kin0_d = nc.dram_tensor("kin0", [P, F0 * T0 * 3], f16,
                            kind="ExternalOutput")
    kin1_d = nc.dram_tensor("kin1", [P, F1 * T1 * 3], f16,
                            kind="ExternalOutput")

    def apx(tl, off, *dims):
        t = tl[:] if not isinstance(tl, AP) else tl
        return AP(t.tensor, t.offset + off,
                  [[t.ap[0][0], P]] + [list(d) for d in dims])

    def compose_1d(E, lanes, a_off, a_step, b_off, b_step, o_off, o_step,
                   tA, tB, a_tile, b_tile, o_tile):
        for k, dst in ((0, tA), (1, tB)):
            E.tensor_mul(
                out=apx(dst, 0, (12, lanes), (4, 3), (1, 4)),
                in0=apx(a_tile, a_off + k, (a_step, lanes), (4, 3), (0, 4)),
                in1=apx(b_tile, b_off + 4 * k, (b_step, lanes), (0, 3), (1, 4)),
            )
        E.tensor_add(
            out=apx(tA, 0, (12, lanes), (1, 12)),
            in0=apx(tA, 0, (12, lanes), (1, 12)),
            in1=apx(tB, 0, (12, lanes), (1, 12)))
        E.tensor_mul(
            out=apx(tB, 0, (12, lanes), (4, 3), (1, 4)),
            in0=apx(a_tile, a_off + 2, (a_step, lanes), (4, 3), (0, 4)),
            in1=apx(b_tile, b_off + 8, (b_step, lanes), (0, 3), (1, 4)),
        )
        E.tensor_add(
            out=apx(o_tile, o_off, (o_step, lanes), (1, 12)),
            in0=apx(tA, 0, (12, lanes), (1, 12)),
            in1=apx(tB, 0, (12, lanes), (1, 12)),
        )
        E.tensor_add(
            out=apx(o_tile, o_off + 3, (o_step, lanes), (4, 3)),
            in0=apx(o_tile, o_off + 3, (o_step, lanes), (4, 3)),
            in1=apx(a_tile, a_off + 3, (a_step, lanes), (4, 3)),
        )

    def excl_blocks(E, SC, CS, U, LPS, spx, lp2, rx, tA, tB):
        SC.copy(out=apx(rx, 0, (U * 12, CS), (1, 12)),
                in_=apx(spx, 0, (12, CS), (1, 12)))
        UM = U - 1
        for i in range(3):
            for k, dst in ((0, tA), (1, tB)):
                E.tensor_mul(
                    out=apx(dst, 4 * i, (96, CS), (12, UM), (1, 4)),
                    in0=apx(spx, 4 * i + k, (12, CS), (0, UM), (0, 4)),
                    in1=apx(lp2, 12 + 4 * k, (LPS, CS), (12, UM), (1, 4)))
            E.tensor_add(
                out=apx(tA, 4 * i, (96, CS), (12, UM), (1, 4)),
                in0=apx(tA, 4 * i, (96, CS), (12, UM), (1, 4)),
                in1=apx(tB, 4 * i, (96, CS), (12, UM), (1, 4)))
            E.tensor_mul(
                out=apx(tB, 4 * i, (96, CS), (12, UM), (1, 4)),
                in0=apx(spx, 4 * i + 2, (12, CS), (0, UM), (0, 4)),
                in1=apx(lp2, 12 + 8, (LPS, CS), (12, UM), (1, 4)))
            E.tensor_add(
                out=apx(rx, 12 + 4 * i, (96, CS), (12, UM), (1, 4)),
                in0=apx(tA, 4 * i, (96, CS), (12, UM), (1, 4)),
                in1=apx(tB, 4 * i, (96, CS), (12, UM), (1, 4)))
        E.tensor_add(
            out=apx(rx, 12 + 3, (96, CS), (12, UM), (4, 3)),
            in0=apx(rx, 12 + 3, (96, CS), (12, UM), (4, 3)),
            in1=apx(spx, 3, (12, CS), (0, UM), (4, 3)))

    with tile.TileContext(nc) as tc:
      for _rep in range(repeat):
        with tc.tile_pool(name="main", bufs=1) as mp:
            X0 = mp.tile([P, T0 * F0 * 9], f16)
            w0 = mp.tile([P, T0 * F0 * 3], f16)
            tW1_0 = mp.tile([P, T0 * F0], f16)
            tW2_0 = mp.tile([P, T0 * F0], f16)
            tA0 = mp.tile([P, 4 * F0], f16)
            tB0 = mp.tile([P, 6 * F0], f16)
            tC0 = mp.tile([P, 6 * F0], f16)
            tg0 = {nm: mp.tile([P, T0 * F0], f16, name=f"tg0_{nm}")
                   for nm in ("sa", "ca", "st", "ct")}
            apl0 = mp.tile([P, CHI * L0], f32)
            wsc0 = mp.tile([P, CHI * L0], f32)
            aw0 = mp.tile([P, CHI * L0], f32)
            d16_0 = mp.tile([P, T0 * F0], f16)
            tg1 = {nm: mp.tile([P, T1 * F1], f16, name=f"tg1_{nm}")
                   for nm in ("sa", "ca", "st", "ct")}
            apl1 = mp.tile([P, CHI * L1], f32)
            wsc1 = mp.tile([P, CHI * L1], f32)
            aw1 = mp.tile([P, CHI * L1], f32)
            d16_1 = mp.tile([P, T1 * F1], f16)
            bht0 = mp.tile([P, F0 * 12], f32)
            rx0 = mp.tile([P, F0 * 12], f32)
            rx16_0 = mp.tile([P, F0 * 12], f16)
            lp2_0 = mp.tile([P, CHI * S0 * (U0 + 1) * 12], f32)
            spx0 = mp.tile([P, CHI * S0 * 12], f32)
            lp2_1 = mp.tile([P, CHI * S1 * (U1 + 1) * 12], f32)
            spx1 = mp.tile([P, CHI * S1 * 12], f32)
            tAh = mp.tile([P, 96 * CHI * S0], f32)
            tBh = mp.tile([P, 96 * CHI * S0], f32)
            a32t = mp.tile([P, CHI * 12], f32)
            rbr = mp.tile([P, CHI * 12], f32)
            jd = mp.tile([P, CHI * 9], f32)
            jang = mp.tile([P, CHI * 2 * 3], f32)
            jsin = mp.tile([P, CHI * 2 * 3], f32)
            jcos = mp.tile([P, CHI * 2 * 3], f32)
            re_ = mp.tile([P, CHI * 2 * 9], f32)
            rj = mp.tile([P, CHI * 9], f32)
            jtmp = mp.tile([P, CHI * 2 * 9], f32)
            halfpi = mp.tile([P, 1], f32)

            V = nc.vector
            SC = nc.scalar

            nc.sync.dma_start(out=jd[:], in_=jd_d[:])
            V.memset(halfpi[:], PI / 2)

            # prefill identities (fp32 hierarchy tiles)
            V.memset(lp2_0[:], 0.0)
            V.memset(apx(lp2_0, 0, ((U0 + 1) * 12, CHI * S0), (5, 3)), 1.0)
            V.memset(spx0[:], 0.0)
            V.memset(apx(spx0, 0, (S0 * 12, CHI), (5, 3)), 1.0)
            V.memset(lp2_1[:], 0.0)
            V.memset(apx(lp2_1, 0, ((U1 + 1) * 12, CHI * S1), (5, 3)), 1.0)
            V.memset(apx(X0, 2 * F0, (1, F0)), 0.0)  # slab0 e=2 plane

            # ---- JUMP HT build (fp32, tiny) ----
            V.tensor_copy(out=jang[:], in_=apx(jd, 3, (9, CHI), (3, 2),
                                               (1, 3)))
            V.add_range_wrap(out=jsin[:], in_=jang[:], shift=0.0,
                             bound=PI, period=2 * PI)
            SC.activation(out=jsin[:], in_=jsin[:], func=SIN)
            V.add_range_wrap(out=jcos[:], in_=jang[:], shift=PI / 2,
                             bound=PI, period=2 * PI)
            SC.activation(out=jcos[:], in_=jcos[:], func=SIN)

            CR = CHI * 2

            def sc_(tl, ang):
                return apx(tl, ang, (3, CR))

            def re(e):
                return apx(re_, e, (9, CR))

            def jt1(e):
                return apx(jtmp, e, (9, CR))

            sa_ = lambda: sc_(jsin, 0)
            sb = lambda: sc_(jsin, 1)
            s_c = lambda: sc_(jsin, 2)
            ca_ = lambda: sc_(jcos, 0)
            cb = lambda: sc_(jcos, 1)
            c_c = lambda: sc_(jcos, 2)
            V.tensor_mul(out=re(0), in0=c_c(), in1=cb())
            V.tensor_mul(out=jt1(0), in0=sb(), in1=sa_())
            V.tensor_mul(out=jt1(1), in0=sb(), in1=ca_())
            V.tensor_mul(out=jt1(2), in0=c_c(), in1=jt1(0))
            V.tensor_mul(out=jt1(3), in0=s_c(), in1=ca_())
            V.tensor_sub(out=re(1), in0=jt1(2), in1=jt1(3))
            V.tensor_mul(out=jt1(2), in0=c_c(), in1=jt1(1))
            V.tensor_mul(out=jt1(3), in0=s_c(), in1=sa_())
            V.tensor_add(out=re(2), in0=jt1(2), in1=jt1(3))
            V.tensor_mul(out=re(3), in0=s_c(), in1=cb())
            V.tensor_mul(out=jt1(2), in0=s_c(), in1=jt1(0))
            V.tensor_mul(out=jt1(3), in0=c_c(), in1=ca_())
            V.tensor_add(out=re(4), in0=jt1(2), in1=jt1(3))
            V.tensor_mul(out=jt1(2), in0=s_c(), in1=jt1(1))
            V.tensor_mul(out=jt1(3), in0=c_c(), in1=sa_())
            V.tensor_sub(out=re(5), in0=jt1(2), in1=jt1(3))
            V.tensor_scalar_mul(out=re(6), in0=sb(), scalar1=-1.0)
            V.tensor_mul(out=re(7), in0=cb(), in1=sa_())
            V.tensor_mul(out=re(8), in0=cb(), in1=ca_())
            V.tensor_mul(
                out=apx(rj, 0, (9, CHI), (3, 3), (1, 3)),
                in0=apx(re_, 0, (18, CHI), (3, 3), (0, 3)),
                in1=apx(re_, 9, (18, CHI), (0, 3), (1, 3)))
            V.tensor_mul(
                out=apx(jtmp, 0, (9, CHI), (3, 3), (1, 3)),
                in0=apx(re_, 1, (18, CHI), (3, 3), (0, 3)),
                in1=apx(re_, 12, (18, CHI), (0, 3), (1, 3)))
            V.tensor_add(out=rj[:, : CHI * 9], in0=rj[:, : CHI * 9],
                         in1=jtmp[:, : CHI * 9])
            V.tensor_mul(
                out=apx(jtmp, 0, (9, CHI), (3, 3), (1, 3)),
                in0=apx(re_, 2, (18, CHI), (3, 3), (0, 3)),
                in1=apx(re_, 15, (18, CHI), (0, 3), (1, 3)))
            V.tensor_add(out=rj[:, : CHI * 9], in0=rj[:, : CHI * 9],
                         in1=jtmp[:, : CHI * 9])

            # ======== trig (both gens): t-major fp32 inputs ========
            # host pre-transposes theta/alpha/d to device order (t-major)
            # and pre-folds alpha = phi_c(parent) + phi_p (incl. the branch
            # root fold), so trig is wrap + contiguous ACT sines only.
            def emit_trig(tht, alt, tg, wsc, aw, F, T):
                V.add_range_wrap(out=wsc[:], in_=tht[:], shift=0.0,
                                 bound=PI, period=2 * PI)
                SC.activation(out=tg["st"][:], in_=wsc[:], func=SIN)
                SC.activation(out=aw[:], in_=wsc[:], func=ABS)
                SC.activation(out=tg["ct"][:], in_=aw[:], func=SIN,
                              scale=-1.0, bias=halfpi[:])
                V.add_range_wrap(out=wsc[:], in_=alt[:], shift=0.0,
                                 bound=PI, period=2 * PI)
                SC.activation(out=tg["sa"][:], in_=wsc[:], func=SIN)
                SC.activation(out=aw[:], in_=wsc[:], func=ABS)
                SC.activation(out=tg["ca"][:], in_=aw[:], func=SIN,
                              scale=-1.0, bias=halfpi[:])

            def emit_fold(X, tg, F, T):
                def tp(nm):
                    return apx(tg[nm], 0, (F, T), (1, F))

                def xo(e):
                    return apx(X, e * F, (9 * F, T), (1, F))

                SC.activation(out=xo(0), in_=tp("ct"), func=CPY, scale=-1.0)
                SC.activation(out=xo(1), in_=tp("st"), func=CPY, scale=-1.0)
                SC.activation(out=xo(5), in_=tp("sa"), func=CPY, scale=-1.0)
                SC.activation(out=xo(8), in_=tp("ca"), func=CPY)
                V.tensor_mul(out=xo(3), in0=tp("ca"), in1=tp("st"))
                V.tensor_mul(out=xo(4), in0=tp("ca"), in1=xo(0))
                V.tensor_mul(out=xo(6), in0=tp("sa"), in1=tp("st"))
                V.tensor_mul(out=xo(7), in0=tp("sa"), in1=xo(0))

            with tc.tile_pool(name="pdof", bufs=1) as pd:
                th0t = pd.tile([P, CHI * L0], f32)
                al0t = pd.tile([P, CHI * L0], f32)
                dt0t = pd.tile([P, CHI * L0], f32)
                th1t = pd.tile([P, CHI * L1], f32)
                al1t = pd.tile([P, CHI * L1], f32)
                dt1t = pd.tile([P, CHI * L1], f32)

                nc.sync.dma_start(out=th0t[:], in_=th0_d[:])
                nc.sync.dma_start(out=al0t[:], in_=al0_d[:])
                nc.sync.dma_start(out=dt0t[:], in_=dt0_d[:])
                nc.sync.dma_start(out=th1t[:], in_=th1_d[:])
                nc.sync.dma_start(out=al1t[:], in_=al1_d[:])
                nc.sync.dma_start(out=dt1t[:], in_=dt1_d[:])

                emit_trig(th0t, al0t, tg0, wsc0, aw0, F0, T0)
                SC.activation(out=d16_0[:], in_=dt0t[:], func=CPY)
                emit_fold(X0, tg0, F0, T0)
                V.tensor_copy(out=apx(X0, 0, (F0, 9), (J0, CHI)),
                              in_=apx(rj, 0, (1, 9), (9, CHI)))
                emit_trig(th1t, al1t, tg1, wsc1, aw1, F1, T1)
                SC.activation(out=d16_1[:], in_=dt1t[:], func=CPY)

            # ======== scans + rest ========
            with tc.tile_pool(name="px1", bufs=1) as px:
                X1 = px.tile([P, T1 * F1 * 9], f16)
                w1 = px.tile([P, T1 * F1 * 3], f16)
                tW1_1 = px.tile([P, T1 * F1], f16)
                tW2_1 = px.tile([P, T1 * F1], f16)
                tA1 = px.tile([P, 4 * F1], f16)
                tB1 = px.tile([P, 6 * F1], f16)
                tC1 = px.tile([P, 6 * F1], f16)
                bht1 = px.tile([P, F1 * 12], f32)
                rx1 = px.tile([P, F1 * 12], f32)
                rx16_1 = px.tile([P, F1 * 12], f16)
                tA1h = px.tile([P, 96 * CHI * S1], f32)
                tB1h = px.tile([P, 96 * CHI * S1], f32)

                V.memset(apx(X1, 2 * F1, (1, F1)), 0.0)
                emit_fold(X1, tg1, F1, T1)

                def scan_i(X, tA, tB, tC, F, t, i):
                    pb = (t - 1) * 9 * F
                    cb = t * 9 * F
                    if i == 0:
                        V.tensor_mul(
                            out=apx(tA, 0, (2 * F, 2), (F, 2), (1, F)),
                            in0=apx(X, pb, (3 * F, 2), (0, 2), (1, F)),
                            in1=apx(X, cb, (0, 2), (F, 2), (1, F)))
                    elif i == 1:
                        V.tensor_mul(
                            out=apx(tB, 0, (3 * F, 2), (F, 3), (1, F)),
                            in0=apx(X, pb + F, (3 * F, 2), (0, 3), (1, F)),
                            in1=apx(X, cb + 3 * F, (0, 2), (F, 3), (1, F)))
                    elif i == 2:
                        V.tensor_mul(
                            out=apx(tC, 0, (3 * F, 2), (F, 3), (1, F)),
                            in0=apx(X, pb + 2 * F, (3 * F, 2), (0, 3),
                                    (1, F)),
                            in1=apx(X, cb + 6 * F, (0, 2), (F, 3), (1, F)))
                    elif i == 3:
                        V.tensor_add(
                            out=apx(tA, 0, (2 * F, 2), (F, 2), (1, F)),
                            in0=apx(tA, 0, (2 * F, 2), (F, 2), (1, F)),
                            in1=apx(tB, 0, (3 * F, 2), (F, 2), (1, F)))
                    elif i == 4:
                        V.tensor_add(
                            out=apx(X, cb, (3 * F, 2), (F, 2), (1, F)),
                            in0=apx(tA, 0, (2 * F, 2), (F, 2), (1, F)),
                            in1=apx(tC, 0, (3 * F, 2), (F, 2), (1, F)))
                    else:
                        V.tensor_add(
                            out=apx(X, cb + 2 * F, (3 * F, 2), (1, F)),
                            in0=apx(tB, 2 * F, (3 * F, 2), (1, F)),
                            in1=apx(tC, 2 * F, (3 * F, 2), (1, F)))

                for t in range(1, T0):
                    for i in range(6):
                        scan_i(X0, tA0, tB0, tC0, F0, t, i)
                        if t < T1:
                            scan_i(X1, tA1, tB1, tC1, F1, t, i)

                def emit_w(X, w, tW1, tW2, d16, F, T):
                    V.tensor_mul(out=apx(tW1, 0, (F, T), (1, F)),
                                 in0=apx(X, F, (9 * F, T), (1, F)),
                                 in1=apx(X, 5 * F, (9 * F, T), (1, F)))
                    V.tensor_mul(out=apx(tW2, 0, (F, T), (1, F)),
                                 in0=apx(X, 2 * F, (9 * F, T), (1, F)),
                                 in1=apx(X, 4 * F, (9 * F, T), (1, F)))
                    V.tensor_sub(out=apx(tW1, 0, (F, T), (1, F)),
                                 in0=apx(tW1, 0, (F, T), (1, F)),
                                 in1=apx(tW2, 0, (F, T), (1, F)))
                    V.tensor_mul(out=apx(w, 2 * F, (3 * F, T), (1, F)),
                                 in0=apx(tW1, 0, (F, T), (1, F)),
                                 in1=apx(d16, 0, (F, T), (1, F)))
                    V.tensor_mul(out=apx(w, 0, (3 * F, T), (F, 2), (1, F)),
                                 in0=apx(X, 0, (9 * F, T), (3 * F, 2),
                                         (1, F)),
                                 in1=apx(d16, 0, (F, T), (0, 2), (1, F)))

                emit_w(X0, w0, tW1_0, tW2_0, d16_0, F0, T0)
                emit_w(X1, w1, tW1_1, tW2_1, d16_1, F1, T1)

                # jump translation into w0 slab0 lanes chi*J0
                V.tensor_copy(out=apx(w0, 0, (F0, 3), (J0, CHI)),
                              in_=apx(jd, 0, (1, 3), (9, CHI)))

                # a32: in-block HT of branch root (lane j=32 per chi, t=0)
                V.tensor_copy(out=apx(a32t, 0, (12, CHI), (4, 2), (1, 3)),
                              in_=apx(X0, 32, (J0, CHI), (3 * F0, 2),
                                      (F0, 3)))
                SC.copy(out=apx(a32t, 8, (12, CHI)),
                        in_=apx(tW1_0, 32, (J0, CHI)))
                for dsti, (e1, e2), (e3, e4) in ((9, (2, 3), (0, 5)),
                                                 (10, (0, 4), (1, 3))):
                    V.tensor_mul(out=apx(tAh, 0, (1, CHI)),
                                 in0=apx(X0, 32 + e1 * F0, (J0, CHI)),
                                 in1=apx(X0, 32 + e2 * F0, (J0, CHI)))
                    V.tensor_mul(out=apx(tBh, 0, (1, CHI)),
                                 in0=apx(X0, 32 + e3 * F0, (J0, CHI)),
                                 in1=apx(X0, 32 + e4 * F0, (J0, CHI)))
                    V.tensor_sub(out=apx(a32t, dsti, (12, CHI)),
                                 in0=apx(tAh, 0, (1, CHI)),
                                 in1=apx(tBh, 0, (1, CHI)))

                # cumsums (slab-contiguous fp16)
                for t in range(1, T0):
                    V.tensor_add(
                        out=apx(w0, t * 3 * F0, (1, 3 * F0)),
                        in0=apx(w0, t * 3 * F0, (1, 3 * F0)),
                        in1=apx(w0, (t - 1) * 3 * F0, (1, 3 * F0)))
                    if t < T1:
                        V.tensor_add(
                            out=apx(w1, t * 3 * F1, (1, 3 * F1)),
                            in0=apx(w1, t * 3 * F1, (1, 3 * F1)),
                            in1=apx(w1, (t - 1) * 3 * F1, (1, 3 * F1)))

                # a32 translation (slab 0 of cumsum = w slab 0)
                V.tensor_copy(out=apx(a32t, 3, (12, CHI), (4, 3)),
                              in_=apx(w0, 32, (J0, CHI), (F0, 3)))

                # block-total HTs -> fp32 packed bht
                def emit_bht(X, w, tW1, bht, F, T):
                    base = (T - 1) * 9 * F
                    SC.copy(out=apx(bht, 0, (12, F), (4, 2), (1, 3)),
                            in_=apx(X, base, (1, F), (3 * F, 2), (F, 3)))
                    SC.copy(out=apx(bht, 8, (12, F)),
                            in_=apx(tW1, (T - 1) * F, (1, F)))
                    for dsti, (e1, e2), (e3, e4) in ((9, (2, 3), (0, 5)),
                                                     (10, (0, 4), (1, 3))):
                        V.tensor_mul(out=apx(tAh, 0, (1, F)),
                                     in0=apx(X, base + e1 * F, (1, F)),
                                     in1=apx(X, base + e2 * F, (1, F)))
                        V.tensor_mul(out=apx(tBh, 0, (1, F)),
                                     in0=apx(X, base + e3 * F, (1, F)),
                                     in1=apx(X, base + e4 * F, (1, F)))
                        V.tensor_sub(out=apx(bht, dsti, (12, F)),
                                     in0=apx(tAh, 0, (1, F)),
                                     in1=apx(tBh, 0, (1, F)))
                    SC.copy(out=apx(bht, 3, (12, F), (4, 3)),
                            in_=apx(w, (T - 1) * 3 * F, (1, F), (F, 3)))

                emit_bht(X0, w0, tW1_0, bht0, F0, T0)
                emit_bht(X1, w1, tW1_1, bht1, F1, T1)

                # ---- hierarchy (fp32, as v1) ----
                LPS0 = (U0 + 1) * 12
                LPS1 = (U1 + 1) * 12
                V.tensor_copy(out=apx(lp2_0, 12, (LPS0, CHI * S0), (1, 12)),
                              in_=apx(bht0, 0, (U0 * 12, CHI * S0), (1, 12)))
                SC.copy(out=apx(lp2_1, 12, (LPS1, CHI * S1), (1, 12)),
                        in_=apx(bht1, 0, (U1 * 12, CHI * S1), (1, 12)))
                for u in range(1, U0):
                    compose_1d(V, CHI * S0,
                               a_off=u * 12, a_step=LPS0,
                               b_off=u * 12, b_step=U0 * 12,
                               o_off=(u + 1) * 12, o_step=LPS0,
                               tA=tAh, tB=tBh,
                               a_tile=lp2_0, b_tile=bht0, o_tile=lp2_0)
                    if u < U1:
                        compose_1d(V, CHI * S1,
                                   a_off=u * 12, a_step=LPS1,
                                   b_off=u * 12, b_step=U1 * 12,
                                   o_off=(u + 1) * 12, o_step=LPS1,
                                   tA=tA1h, tB=tB1h,
                                   a_tile=lp2_1, b_tile=bht1, o_tile=lp2_1)
                for sidx in range(1, S0):
                    compose_1d(V, CHI,
                               a_off=(sidx - 1) * 12, a_step=S0 * 12,
                               b_off=(sidx - 1) * LPS0 + U0 * 12,
                               b_step=S0 * LPS0,
                               o_off=sidx * 12, o_step=S0 * 12,
                               tA=tAh, tB=tBh,
                               a_tile=spx0, b_tile=lp2_0, o_tile=spx0)
                excl_blocks(V, SC, CHI * S0, U0, LPS0, spx0, lp2_0, rx0,
                            tAh, tBh)
                compose_1d(V, CHI,
                           a_off=32 * 12, a_step=J0 * 12,
                           b_off=0, b_step=12,
                           o_off=0, o_step=12,
                           tA=tAh, tB=tBh,
                           a_tile=rx0, b_tile=a32t, o_tile=rbr)
                SC.copy(out=apx(spx1, 0, (S1 * 12, CHI), (1, 12)),
                        in_=apx(rbr, 0, (12, CHI), (1, 12)))
                # rx -> planar fp16 for the down transform
                SC.copy(out=apx(rx16_0, 0, (1, F0), (F0, 12)),
                        in_=apx(rx0, 0, (12, F0), (1, 12)))

                def down_i(w, rx16, X, tmpoff, F, T, i):
                    xyz = apx(X, 0, (3 * F, T), (F, 3), (1, F))
                    tmp = apx(X, tmpoff, (3 * F, T), (F, 3), (1, F))

                    def rxk(k):
                        return apx(rx16, k * F, (0, T), (4 * F, 3), (1, F))

                    def wk(k):
                        return apx(w, k * F, (3 * F, T), (0, 3), (1, F))

                    if i == 0:
                        V.tensor_mul(out=xyz, in0=rxk(0), in1=wk(0))
                    elif i == 1:
                        V.tensor_mul(out=tmp, in0=rxk(1), in1=wk(1))
                    elif i == 2:
                        V.tensor_add(out=xyz, in0=xyz, in1=tmp)
                    elif i == 3:
                        V.tensor_mul(out=tmp, in0=rxk(2), in1=wk(2))
                    elif i == 4:
                        V.tensor_add(out=xyz, in0=xyz, in1=tmp)
                    else:
                        V.tensor_add(out=xyz, in0=xyz, in1=rxk(3))

                # gen1 level-3 + excl first (covers the rx16_0 cast on ACT),
                # then the down-transforms; xyz stays planar for the DMA and
                # the host undoes the layout.
                for sidx in range(1, S1):
                    compose_1d(V, CHI,
                               a_off=(sidx - 1) * 12, a_step=S1 * 12,
                               b_off=(sidx - 1) * LPS1 + U1 * 12,
                               b_step=S1 * LPS1,
                               o_off=sidx * 12, o_step=S1 * 12,
                               tA=tA1h, tB=tB1h,
                               a_tile=spx1, b_tile=lp2_1, o_tile=spx1)
                excl_blocks(V, SC, CHI * S1, U1, LPS1, spx1, lp2_1, rx1,
                            tA1h, tB1h)
                SC.copy(out=apx(rx16_1, 0, (1, F1), (F1, 12)),
                        in_=apx(rx1, 0, (12, F1), (1, 12)))
                for i in range(6):
                    down_i(w0, rx16_0, X0, 3 * F0 * T0, F0, T0, i)
                nc.sync.dma_start(
                    out=AP(kin0_d, 0, [[F0 * T0 * 3, P], [1, F0 * T0 * 3]]),
                    in_=apx(X0, 0, (1, F0 * T0 * 3)))
                for i in range(6):
                    down_i(w1, rx16_1, X1, 3 * F1 * T1, F1, T1, i)
                nc.sync.dma_start(
                    out=AP(kin1_d, 0, [[F1 * T1 * 3, P], [1, F1 * T1 * 3]]),
                    in_=apx(X1, 0, (1, F1 * T1 * 3)))

    nc.compile()
    return nc


def get_program(repeat=1):
    key = ("nc", repeat)
    if key not in _CACHE:
        _CACHE[key] = _build_program(repeat)
    return _CACHE[key]


# ------------------------------------------------------------------- host
def _shard_inputs(dofs, doftype):
    """Per-core inputs, pre-transposed to device t-major lane order.

    Device order per partition p: index t*F + chi*J + j for atom
    (chi, j, t); host layout [P, CHI*L].  Alpha is pre-folded on the host:
    alpha_p = phi_c(parent) + phi_p(p) (chain starts: phi_p only; branch
    roots fold phi_c of gen0 atom 384)."""
    def to_dev(arr, J, T):
        # arr: [C_core, L] (chain-major) -> [P, T*CHI*J]
        a = arr.reshape(CHI, P, J, T)
        return np.ascontiguousarray(
            a.transpose(1, 3, 0, 2).reshape(P, CHI * J * T))

    chain_starts = 1 + np.arange(C0, dtype=np.int64) * L0
    jd_all = np.ascontiguousarray(dofs[chain_starts])       # [C0, 9]

    ph0 = dofs[1:BOFF, 0].reshape(C0, L0)
    th0 = dofs[1:BOFF, 1].reshape(C0, L0)
    d0 = dofs[1:BOFF, 2].reshape(C0, L0)
    pc0 = dofs[1:BOFF, 3].reshape(C0, L0)
    al0 = np.empty_like(ph0)
    al0[:, 0] = 0.0
    al0[:, 1] = ph0[:, 1]
    al0[:, 2:] = ph0[:, 2:] + pc0[:, 1:-1]

    ph1 = dofs[BOFF:, 0].reshape(C1, L1)
    th1 = dofs[BOFF:, 1].reshape(C1, L1)
    d1 = dofs[BOFF:, 2].reshape(C1, L1)
    pc1 = dofs[BOFF:, 3].reshape(C1, L1)
    al1 = np.empty_like(ph1)
    al1[:, 0] = ph1[:, 0] + pc0[:, 384]
    al1[:, 1:] = ph1[:, 1:] + pc1[:, :-1]

    in_maps = []
    for core in range(NCORES):
        s0 = slice(core * CH0, (core + 1) * CH0)
        s1 = slice(core * CH1, (core + 1) * CH1)
        jd = np.ascontiguousarray(
            jd_all[s0].reshape(CHI, P, 9).transpose(1, 0, 2)
            .reshape(P, CHI * 9))
        in_maps.append({
            "th0": to_dev(th0[s0], J0, T0),
            "al0": to_dev(al0[s0], J0, T0),
            "dt0": to_dev(d0[s0], J0, T0),
            "th1": to_dev(th1[s1], J1, T1),
            "al1": to_dev(al1[s1], J1, T1),
            "dt1": to_dev(d1[s1], J1, T1),
            "jd": jd,
        })
    return in_maps


def _lane_ids(id_idx, core):
    """id_idx values of this core's atoms in device lane order (p, f, t)."""
    ids0 = (id_idx[core * A0:(core + 1) * A0]
            .reshape(CHI, P, L0).transpose(1, 0, 2).ravel())
    ids1 = (id_idx[BOFF - 1 + core * A1: BOFF - 1 + (core + 1) * A1]
            .reshape(CHI, P, L1).transpose(1, 0, 2).ravel())
    return ids0, ids1


def _structure_ok(doftype, gen0_paths, gen1_paths):
    chain_starts = 1 + np.arange(C0, dtype=np.int64) * L0
    g0 = np.concatenate(
        [np.zeros((C0, 1), np.int64), chain_starts[:, None] + np.arange(L0)],
        axis=1)
    if not np.array_equal(gen0_paths, g0.astype(gen0_paths.dtype)):
        return False
    branch_roots = chain_starts + L0 // 2
    g1 = np.concatenate(
        [branch_roots[:, None],
         BOFF + (np.arange(C1, dtype=np.int64) * L1)[:, None] + np.arange(L1)],
        axis=1)
    if not np.array_equal(gen1_paths, g1.astype(gen1_paths.dtype)):
        return False
    if doftype[0] != 0:
        return False
    if not np.all(doftype[chain_starts] == 1):
        return False
    dt = doftype.copy()
    dt[chain_starts] = 2
    if not np.all(dt[1:] == 2):
        return False
    return True


def _numpy_fallback(dofs, doftype, gen0_paths, gen1_paths, id_idx):
    """Exact numpy port of the reference (slow path, safety net)."""
    def rx(a):
        c, s = np.cos(a), np.sin(a)
        o, z = np.ones_like(a), np.zeros_like(a)
        return np.stack([np.stack([o, z, z, z], -1), np.stack([z, c, -s, z], -1),
                         np.stack([z, s, c, z], -1), np.stack([z, z, z, o], -1)], -2)

    def ry(a):
        c, s = np.cos(a), np.sin(a)
        o, z = np.ones_like(a), np.zeros_like(a)
        return np.stack([np.stack([c, z, s, z], -1), np.stack([z, o, z, z], -1),
                         np.stack([-s, z, c, z], -1), np.stack([z, z, z, o], -1)], -2)

    def rz(a):
        c, s = np.cos(a), np.sin(a)
        o, z = np.ones_like(a), np.zeros_like(a)
        return np.stack([np.stack([c, -s, z, z], -1), np.stack([s, c, z, z], -1),
                         np.stack([z, z, o, z], -1), np.stack([z, z, z, o], -1)], -2)

    def trans(x, y, z):
        o, zr = np.ones_like(x), np.zeros_like(x)
        return np.stack([np.stack([o, zr, zr, x], -1), np.stack([zr, o, zr, y], -1),
                         np.stack([zr, zr, o, z], -1), np.stack([zr, zr, zr, o], -1)], -2)

    dofs = dofs.astype(np.float32)
    phi_p, theta, d, phi_c = dofs[:, 0], dofs[:, 1], dofs[:, 2], dofs[:, 3]
    z = np.zeros_like(d)
    bond = rx(phi_p) @ rz(np.pi - theta) @ trans(d, z, z) @ rx(phi_c)
    rot = lambda a, b, c: rz(c) @ ry(b) @ rx(a)
    jump = (trans(dofs[:, 0], dofs[:, 1], dofs[:, 2])
            @ rot(dofs[:, 3], dofs[:, 4], dofs[:, 5])
            @ rot(dofs[:, 6], dofs[:, 7], dofs[:, 8]))
    eye = np.broadcast_to(np.eye(4, dtype=dofs.dtype), bond.shape)
    dt = doftype[:, None, None]
    hts = np.where(dt == 1, jump, np.where(dt == 2, bond, eye)).astype(np.float32)
    for paths in (gen0_paths, gen1_paths):
        seg = hts[paths]
        out = np.empty_like(seg)
        out[:, 0] = seg[:, 0]
        for i in range(1, seg.shape[1]):
            out[:, i] = out[:, i - 1] @ seg[:, i]
        hts[paths] = out
    kincoords = hts[:, :3, 3]
    coords = np.zeros((N - 1, 3), dtype=dofs.dtype)
    coords[np.asarray(id_idx)] = kincoords[1:]
    return coords


def kernel(dofs, doftype, gen0_paths, gen1_paths, id_idx):
    dofs = np.asarray(dofs, dtype=np.float32)
    doftype = np.asarray(doftype, dtype=np.int32)
    gen0_paths = np.asarray(gen0_paths)
    gen1_paths = np.asarray(gen1_paths)
    id_idx = np.asarray(id_idx, dtype=np.int32)

    if not _structure_ok(doftype, gen0_paths, gen1_paths):
        return _numpy_fallback(dofs, doftype, gen0_paths, gen1_paths, id_idx)

    from concourse.bass_utils import run_bass_kernel_spmd

    nc = get_program()
    in_maps = _shard_inputs(dofs, doftype)
    res = run_bass_kernel_spmd(nc, in_maps, core_ids=list(range(NCORES)))
    out = np.empty((N - 1, 3), dtype=np.float32)
    for core in range(NCORES):
        ids0, ids1 = _lane_ids(id_idx, core)
        k0 = res.results[core]["kin0"].astype(np.float32)
        k0 = k0.reshape(P, T0, 3, F0).transpose(0, 3, 1, 2).reshape(-1, 3)
        k1 = res.results[core]["kin1"].astype(np.float32)
        k1 = k1.reshape(P, T1, 3, F1).transpose(0, 3, 1, 2).reshape(-1, 3)
        out[ids0] = k0
        out[ids1] = k1
    return out


# revision 6
# speedup vs baseline: 1.8265x; 1.2659x over previous
"""Trainium2 Bass kernel for nn_KinematicOperation — v3: fp16 planar DVE.

Same blocked-scan algorithm as v1, but the lane-parallel phases (bond fold,
level-1 scan, w, cumsum, down-transform) run in fp16 with entry-PLANAR
layouts (inner dim = lanes, unit stride, 4B aligned), which engages the
DVE's 2x_1P mode: tensor_tensor at 2 elem/cycle.  Angles stay fp32 through
the ACT sine (fp16 angle rounding would dominate the error budget); sin/cos
outputs and all downstream per-atom products are fp16.  The block-level
hierarchy (bht, levels, excl) stays fp32 packed as in v1.  ACT does the
transposing casts (trig planes, d-column, rx expansion, output repack).

Layouts per partition (generation with F lanes, T slabs):
  X[t][e][f]   e in 0..8: 3x3 entries row-major; slabs t (fp16)
  w[t][c][f]   c in 0..2 (fp16)
  trig planes nm[t*F + f] (fp16), angles atom-major fp32
  bht/lp2/spx/rx packed 12-elem HTs per lane (fp32), rx16 planar fp16
"""

import os
import sys

import numpy as np

for _p in ("/opt/trn_rl_repo", "/root/.axon_site/_ro/trn_rl_repo"):
    if os.path.isdir(_p) and _p not in sys.path:
        sys.path.insert(0, _p)

C0, L0 = 2048, 768
C1, L1 = 2048, 256
N = 1 + C0 * L0 + C1 * L1
BOFF = 1 + C0 * L0
NCORES = 8
P = 128
CHI = 2
CH0 = C0 // NCORES
CH1 = C1 // NCORES
A0 = CH0 * L0
A1 = CH1 * L1

T0, J0, S0, U0 = 12, 64, 8, 8
F0 = CHI * J0
T1, J1, S1, U1 = 8, 32, 4, 8
F1 = CHI * J1

PI = float(np.pi)

_CACHE = {}


def _build_program(repeat=1):
    from concourse import bacc, mybir, tile
    from concourse.bass import AP

    f32 = mybir.dt.float32
    f16 = mybir.dt.float16
    SIN = mybir.ActivationFunctionType.Sin
    ABS = mybir.ActivationFunctionType.Abs
    CPY = mybir.ActivationFunctionType.Copy

    nc = bacc.Bacc("TRN2", target_bir_lowering=False, debug=False)

    th0_d = nc.dram_tensor("th0", [P, CHI * L0], f32, kind="ExternalInput")
    al0_d = nc.dram_tensor("al0", [P, CHI * L0], f32, kind="ExternalInput")
    dt0_d = nc.dram_tensor("dt0", [P, CHI * L0], f32, kind="ExternalInput")
    th1_d = nc.dram_tensor("th1", [P, CHI * L1], f32, kind="ExternalInput")
    al1_d = nc.dram_tensor("al1", [P, CHI * L1], f32, kind="ExternalInput")
    dt1_d = nc.dram_tensor("dt1", [P, CHI * L1], f32, kind="ExternalInput")
    jd_d = nc.dram_tensor("jd", [P, CHI * 9], f32, kind="ExternalInput")
    kin0_d = nc.dram_tensor("kin0", [P, F0 * T0 * 3], f16,
                            kind="ExternalOutput")
    kin1_d = nc.dram_tensor("kin1", [P, F1 * T1 * 3], f16,
                            kind="ExternalOutput")

    def apx(tl, off, *dims):
        t = tl[:] if not isinstance(tl, AP) else tl
        return AP(t.tensor, t.offset + off,
                  [[t.ap[0][0], P]] + [list(d) for d in dims])

    def compose_1d(E, lanes, a_off, a_step, b_off, b_step, o_off, o_step,
                   tA, tB, a_tile, b_tile, o_tile):
        for k, dst in ((0, tA), (1, tB)):
            E.tensor_mul(
                out=apx(dst, 0, (12, lanes), (4, 3), (1, 4)),
                in0=apx(a_tile, a_off + k, (a_step, lanes), (4, 3), (0, 4)),
                in1=apx(b_tile, b_off + 4 * k, (b_step, lanes), (0, 3), (1, 4)),
            )
        E.tensor_add(
            out=apx(tA, 0, (12, lanes), (1, 12)),
            in0=apx(tA, 0, (12, lanes), (1, 12)),
            in1=apx(tB, 0, (12, lanes), (1, 12)))
        E.tensor_mul(
            out=apx(tB, 0, (12, lanes), (4, 3), (1, 4)),
            in0=apx(a_tile, a_off + 2, (a_step, lanes), (4, 3), (0, 4)),
            in1=apx(b_tile, b_off + 8, (b_step, lanes), (0, 3), (1, 4)),
        )
        E.tensor_add(
            out=apx(o_tile, o_off, (o_step, lanes), (1, 12)),
            in0=apx(tA, 0, (12, lanes), (1, 12)),
            in1=apx(tB, 0, (12, lanes), (1, 12)),
        )
        E.tensor_add(
            out=apx(o_tile, o_off + 3, (o_step, lanes), (4, 3)),
            in0=apx(o_tile, o_off + 3, (o_step, lanes), (4, 3)),
            in1=apx(a_tile, a_off + 3, (a_step, lanes), (4, 3)),
        )

    def excl_blocks(E, SC, CS, U, LPS, spx, lp2, rx, tA, tB):
        SC.copy(out=apx(rx, 0, (U * 12, CS), (1, 12)),
                in_=apx(spx, 0, (12, CS), (1, 12)))
        UM = U - 1
        for i in range(3):
            for k, dst in ((0, tA), (1, tB)):
                E.tensor_mul(
                    out=apx(dst, 4 * i, (96, CS), (12, UM), (1, 4)),
                    in0=apx(spx, 4 * i + k, (12, CS), (0, UM), (0, 4)),
                    in1=apx(lp2, 12 + 4 * k, (LPS, CS), (12, UM), (1, 4)))
            E.tensor_add(
                out=apx(tA, 4 * i, (96, CS), (12, UM), (1, 4)),
                in0=apx(tA, 4 * i, (96, CS), (12, UM), (1, 4)),
                in1=apx(tB, 4 * i, (96, CS), (12, UM), (1, 4)))
            E.tensor_mul(
                out=apx(tB, 4 * i, (96, CS), (12, UM), (1, 4)),
                in0=apx(spx, 4 * i + 2, (12, CS), (0, UM), (0, 4)),
                in1=apx(lp2, 12 + 8, (LPS, CS), (12, UM), (1, 4)))
            E.tensor_add(
                out=apx(rx, 12 + 4 * i, (96, CS), (12, UM), (1, 4)),
                in0=apx(tA, 4 * i, (96, CS), (12, UM), (1, 4)),
                in1=apx(tB, 4 * i, (96, CS), (12, UM), (1, 4)))
        E.tensor_add(
            out=apx(rx, 12 + 3, (96, CS), (12, UM), (4, 3)),
            in0=apx(rx, 12 + 3, (96, CS), (12, UM), (4, 3)),
            in1=apx(spx, 3, (12, CS), (0, UM), (4, 3)))

    with tile.TileContext(nc) as tc:
      for _rep in range(repeat):
        with tc.tile_pool(name="main", bufs=1) as mp:
            X0 = mp.tile([P, T0 * F0 * 9], f16)
            w0 = mp.tile([P, T0 * F0 * 3], f16)
            tW1_0 = mp.tile([P, T0 * F0], f16)
            tW2_0 = mp.tile([P, T0 * F0], f16)
            tA0 = mp.tile([P, 4 * F0], f16)
            tB0 = mp.tile([P, 6 * F0], f16)
            tC0 = mp.tile([P, 6 * F0], f16)
            tg0 = {nm: mp.tile([P, T0 * F0], f16, name=f"tg0_{nm}")
                   for nm in ("sa", "ca", "st", "ct")}
            apl0 = mp.tile([P, CHI * L0], f32)
            wsc0 = mp.tile([P, CHI * L0], f32)
            aw0 = mp.tile([P, CHI * L0], f32)
            d16_0 = mp.tile([P, T0 * F0], f16)
            tg1 = {nm: mp.tile([P, T1 * F1], f16, name=f"tg1_{nm}")
                   for nm in ("sa", "ca", "st", "ct")}
            apl1 = mp.tile([P, CHI * L1], f32)
            wsc1 = mp.tile([P, CHI * L1], f32)
            aw1 = mp.tile([P, CHI * L1], f32)
            d16_1 = mp.tile([P, T1 * F1], f16)
            bht0 = mp.tile([P, F0 * 12], f16)
            rx0 = mp.tile([P, F0 * 12], f16)
            rx16_0 = mp.tile([P, F0 * 12], f16)
            lp2_0 = mp.tile([P, CHI * S0 * (U0 + 1) * 12], f16)
            spx0 = mp.tile([P, CHI * S0 * 12], f16)
            lp2_1 = mp.tile([P, CHI * S1 * (U1 + 1) * 12], f16)
            spx1 = mp.tile([P, CHI * S1 * 12], f16)
            tAh = mp.tile([P, 96 * CHI * S0], f16)
            tBh = mp.tile([P, 96 * CHI * S0], f16)
            a32t = mp.tile([P, CHI * 12], f16)
            rbr = mp.tile([P, CHI * 12], f16)
            jd = mp.tile([P, CHI * 9], f32)
            jang = mp.tile([P, CHI * 2 * 3], f32)
            jsin = mp.tile([P, CHI * 2 * 3], f32)
            jcos = mp.tile([P, CHI * 2 * 3], f32)
            re_ = mp.tile([P, CHI * 2 * 9], f32)
            rj = mp.tile([P, CHI * 9], f32)
            jtmp = mp.tile([P, CHI * 2 * 9], f32)
            halfpi = mp.tile([P, 1], f32)

            V = nc.vector
            SC = nc.scalar

            nc.sync.dma_start(out=jd[:], in_=jd_d[:])
            V.memset(halfpi[:], PI / 2)

            # prefill identities (fp32 hierarchy tiles)
            V.memset(lp2_0[:], 0.0)
            V.memset(apx(lp2_0, 0, ((U0 + 1) * 12, CHI * S0), (5, 3)), 1.0)
            V.memset(spx0[:], 0.0)
            V.memset(apx(spx0, 0, (S0 * 12, CHI), (5, 3)), 1.0)
            V.memset(lp2_1[:], 0.0)
            V.memset(apx(lp2_1, 0, ((U1 + 1) * 12, CHI * S1), (5, 3)), 1.0)
            V.memset(apx(X0, 2 * F0, (1, F0)), 0.0)  # slab0 e=2 plane

            # ---- JUMP HT build (fp32, tiny) ----
            V.tensor_copy(out=jang[:], in_=apx(jd, 3, (9, CHI), (3, 2),
                                               (1, 3)))
            V.add_range_wrap(out=jsin[:], in_=jang[:], shift=0.0,
                             bound=PI, period=2 * PI)
            SC.activation(out=jsin[:], in_=jsin[:], func=SIN)
            V.add_range_wrap(out=jcos[:], in_=jang[:], shift=PI / 2,
                             bound=PI, period=2 * PI)
            SC.activation(out=jcos[:], in_=jcos[:], func=SIN)

            CR = CHI * 2

            def sc_(tl, ang):
                return apx(tl, ang, (3, CR))

            def re(e):
                return apx(re_, e, (9, CR))

            def jt1(e):
                return apx(jtmp, e, (9, CR))

            sa_ = lambda: sc_(jsin, 0)
            sb = lambda: sc_(jsin, 1)
            s_c = lambda: sc_(jsin, 2)
            ca_ = lambda: sc_(jcos, 0)
            cb = lambda: sc_(jcos, 1)
            c_c = lambda: sc_(jcos, 2)
            V.tensor_mul(out=re(0), in0=c_c(), in1=cb())
            V.tensor_mul(out=jt1(0), in0=sb(), in1=sa_())
            V.tensor_mul(out=jt1(1), in0=sb(), in1=ca_())
            V.tensor_mul(out=jt1(2), in0=c_c(), in1=jt1(0))
            V.tensor_mul(out=jt1(3), in0=s_c(), in1=ca_())
            V.tensor_sub(out=re(1), in0=jt1(2), in1=jt1(3))
            V.tensor_mul(out=jt1(2), in0=c_c(), in1=jt1(1))
            V.tensor_mul(out=jt1(3), in0=s_c(), in1=sa_())
            V.tensor_add(out=re(2), in0=jt1(2), in1=jt1(3))
            V.tensor_mul(out=re(3), in0=s_c(), in1=cb())
            V.tensor_mul(out=jt1(2), in0=s_c(), in1=jt1(0))
            V.tensor_mul(out=jt1(3), in0=c_c(), in1=ca_())
            V.tensor_add(out=re(4), in0=jt1(2), in1=jt1(3))
            V.tensor_mul(out=jt1(2), in0=s_c(), in1=jt1(1))
            V.tensor_mul(out=jt1(3), in0=c_c(), in1=sa_())
            V.tensor_sub(out=re(5), in0=jt1(2), in1=jt1(3))
            V.tensor_scalar_mul(out=re(6), in0=sb(), scalar1=-1.0)
            V.tensor_mul(out=re(7), in0=cb(), in1=sa_())
            V.tensor_mul(out=re(8), in0=cb(), in1=ca_())
            V.tensor_mul(
                out=apx(rj, 0, (9, CHI), (3, 3), (1, 3)),
                in0=apx(re_, 0, (18, CHI), (3, 3), (0, 3)),
                in1=apx(re_, 9, (18, CHI), (0, 3), (1, 3)))
            V.tensor_mul(
                out=apx(jtmp, 0, (9, CHI), (3, 3), (1, 3)),
                in0=apx(re_, 1, (18, CHI), (3, 3), (0, 3)),
                in1=apx(re_, 12, (18, CHI), (0, 3), (1, 3)))
            V.tensor_add(out=rj[:, : CHI * 9], in0=rj[:, : CHI * 9],
                         in1=jtmp[:, : CHI * 9])
            V.tensor_mul(
                out=apx(jtmp, 0, (9, CHI), (3, 3), (1, 3)),
                in0=apx(re_, 2, (18, CHI), (3, 3), (0, 3)),
                in1=apx(re_, 15, (18, CHI), (0, 3), (1, 3)))
            V.tensor_add(out=rj[:, : CHI * 9], in0=rj[:, : CHI * 9],
                         in1=jtmp[:, : CHI * 9])

            # ======== trig (both gens): t-major fp32 inputs ========
            # host pre-transposes theta/alpha/d to device order (t-major)
            # and pre-folds alpha = phi_c(parent) + phi_p (incl. the branch
            # root fold), so trig is wrap + contiguous ACT sines only.
            def emit_trig(tht, alt, tg, wsc, aw, F, T):
                V.add_range_wrap(out=wsc[:], in_=tht[:], shift=0.0,
                                 bound=PI, period=2 * PI)
                SC.activation(out=tg["st"][:], in_=wsc[:], func=SIN)
                SC.activation(out=aw[:], in_=wsc[:], func=ABS)
                SC.activation(out=tg["ct"][:], in_=aw[:], func=SIN,
                              scale=-1.0, bias=halfpi[:])
                V.add_range_wrap(out=wsc[:], in_=alt[:], shift=0.0,
                                 bound=PI, period=2 * PI)
                SC.activation(out=tg["sa"][:], in_=wsc[:], func=SIN)
                SC.activation(out=aw[:], in_=wsc[:], func=ABS)
                SC.activation(out=tg["ca"][:], in_=aw[:], func=SIN,
                              scale=-1.0, bias=halfpi[:])

            def emit_fold(X, tg, F, T):
                def tp(nm):
                    return apx(tg[nm], 0, (F, T), (1, F))

                def xo(e):
                    return apx(X, e * F, (9 * F, T), (1, F))

                SC.activation(out=xo(0), in_=tp("ct"), func=CPY, scale=-1.0)
                SC.activation(out=xo(1), in_=tp("st"), func=CPY, scale=-1.0)
                SC.activation(out=xo(5), in_=tp("sa"), func=CPY, scale=-1.0)
                SC.activation(out=xo(8), in_=tp("ca"), func=CPY)
                V.tensor_mul(out=xo(3), in0=tp("ca"), in1=tp("st"))
                V.tensor_mul(out=xo(4), in0=tp("ca"), in1=xo(0))
                V.tensor_mul(out=xo(6), in0=tp("sa"), in1=tp("st"))
                V.tensor_mul(out=xo(7), in0=tp("sa"), in1=xo(0))

            with tc.tile_pool(name="pdof", bufs=1) as pd:
                th0t = pd.tile([P, CHI * L0], f32)
                al0t = pd.tile([P, CHI * L0], f32)
                dt0t = pd.tile([P, CHI * L0], f32)
                th1t = pd.tile([P, CHI * L1], f32)
                al1t = pd.tile([P, CHI * L1], f32)
                dt1t = pd.tile([P, CHI * L1], f32)

                nc.sync.dma_start(out=th0t[:], in_=th0_d[:])
                nc.sync.dma_start(out=al0t[:], in_=al0_d[:])
                nc.sync.dma_start(out=dt0t[:], in_=dt0_d[:])
                nc.sync.dma_start(out=th1t[:], in_=th1_d[:])
                nc.sync.dma_start(out=al1t[:], in_=al1_d[:])
                nc.sync.dma_start(out=dt1t[:], in_=dt1_d[:])

                emit_trig(th0t, al0t, tg0, wsc0, aw0, F0, T0)
                SC.activation(out=d16_0[:], in_=dt0t[:], func=CPY)
                emit_fold(X0, tg0, F0, T0)
                V.tensor_copy(out=apx(X0, 0, (F0, 9), (J0, CHI)),
                              in_=apx(rj, 0, (1, 9), (9, CHI)))
                emit_trig(th1t, al1t, tg1, wsc1, aw1, F1, T1)
                SC.activation(out=d16_1[:], in_=dt1t[:], func=CPY)

            # ======== scans + rest ========
            with tc.tile_pool(name="px1", bufs=1) as px:
                X1 = px.tile([P, T1 * F1 * 9], f16)
                w1 = px.tile([P, T1 * F1 * 3], f16)
                tW1_1 = px.tile([P, T1 * F1], f16)
                tW2_1 = px.tile([P, T1 * F1], f16)
                tA1 = px.tile([P, 4 * F1], f16)
                tB1 = px.tile([P, 6 * F1], f16)
                tC1 = px.tile([P, 6 * F1], f16)
                bht1 = px.tile([P, F1 * 12], f16)
                rx1 = px.tile([P, F1 * 12], f16)
                rx16_1 = px.tile([P, F1 * 12], f16)
                tA1h = px.tile([P, 96 * CHI * S1], f16)
                tB1h = px.tile([P, 96 * CHI * S1], f16)

                V.memset(apx(X1, 2 * F1, (1, F1)), 0.0)
                emit_fold(X1, tg1, F1, T1)

                def scan_i(X, tA, tB, tC, F, t, i):
                    pb = (t - 1) * 9 * F
                    cb = t * 9 * F
                    if i == 0:
                        V.tensor_mul(
                            out=apx(tA, 0, (2 * F, 2), (F, 2), (1, F)),
                            in0=apx(X, pb, (3 * F, 2), (0, 2), (1, F)),
                            in1=apx(X, cb, (0, 2), (F, 2), (1, F)))
                    elif i == 1:
                        V.tensor_mul(
                            out=apx(tB, 0, (3 * F, 2), (F, 3), (1, F)),
                            in0=apx(X, pb + F, (3 * F, 2), (0, 3), (1, F)),
                            in1=apx(X, cb + 3 * F, (0, 2), (F, 3), (1, F)))
                    elif i == 2:
                        V.tensor_mul(
                            out=apx(tC, 0, (3 * F, 2), (F, 3), (1, F)),
                            in0=apx(X, pb + 2 * F, (3 * F, 2), (0, 3),
                                    (1, F)),
                            in1=apx(X, cb + 6 * F, (0, 2), (F, 3), (1, F)))
                    elif i == 3:
                        V.tensor_add(
                            out=apx(tA, 0, (2 * F, 2), (F, 2), (1, F)),
                            in0=apx(tA, 0, (2 * F, 2), (F, 2), (1, F)),
                            in1=apx(tB, 0, (3 * F, 2), (F, 2), (1, F)))
                    elif i == 4:
                        V.tensor_add(
                            out=apx(X, cb, (3 * F, 2), (F, 2), (1, F)),
                            in0=apx(tA, 0, (2 * F, 2), (F, 2), (1, F)),
                            in1=apx(tC, 0, (3 * F, 2), (F, 2), (1, F)))
                    else:
                        V.tensor_add(
                            out=apx(X, cb + 2 * F, (3 * F, 2), (1, F)),
                            in0=apx(tB, 2 * F, (3 * F, 2), (1, F)),
                            in1=apx(tC, 2 * F, (3 * F, 2), (1, F)))

                for t in range(1, T0):
                    for i in range(6):
                        scan_i(X0, tA0, tB0, tC0, F0, t, i)
                        if t < T1:
                            scan_i(X1, tA1, tB1, tC1, F1, t, i)

                def emit_w(X, w, tW1, tW2, d16, F, T):
                    V.tensor_mul(out=apx(tW1, 0, (F, T), (1, F)),
                                 in0=apx(X, F, (9 * F, T), (1, F)),
                                 in1=apx(X, 5 * F, (9 * F, T), (1, F)))
                    V.tensor_mul(out=apx(tW2, 0, (F, T), (1, F)),
                                 in0=apx(X, 2 * F, (9 * F, T), (1, F)),
                                 in1=apx(X, 4 * F, (9 * F, T), (1, F)))
                    V.tensor_sub(out=apx(tW1, 0, (F, T), (1, F)),
                                 in0=apx(tW1, 0, (F, T), (1, F)),
                                 in1=apx(tW2, 0, (F, T), (1, F)))
                    V.tensor_mul(out=apx(w, 2 * F, (3 * F, T), (1, F)),
                                 in0=apx(tW1, 0, (F, T), (1, F)),
                                 in1=apx(d16, 0, (F, T), (1, F)))
                    V.tensor_mul(out=apx(w, 0, (3 * F, T), (F, 2), (1, F)),
                                 in0=apx(X, 0, (9 * F, T), (3 * F, 2),
                                         (1, F)),
                                 in1=apx(d16, 0, (F, T), (0, 2), (1, F)))

                emit_w(X0, w0, tW1_0, tW2_0, d16_0, F0, T0)
                emit_w(X1, w1, tW1_1, tW2_1, d16_1, F1, T1)

                # jump translation into w0 slab0 lanes chi*J0
                V.tensor_copy(out=apx(w0, 0, (F0, 3), (J0, CHI)),
                              in_=apx(jd, 0, (1, 3), (9, CHI)))

                # a32: in-block HT of branch root (lane j=32 per chi, t=0)
                V.tensor_copy(out=apx(a32t, 0, (12, CHI), (4, 2), (1, 3)),
                              in_=apx(X0, 32, (J0, CHI), (3 * F0, 2),
                                      (F0, 3)))
                SC.copy(out=apx(a32t, 8, (12, CHI)),
                        in_=apx(tW1_0, 32, (J0, CHI)))
                for dsti, (e1, e2), (e3, e4) in ((9, (2, 3), (0, 5)),
                                                 (10, (0, 4), (1, 3))):
                    V.tensor_mul(out=apx(tAh, 0, (1, CHI)),
                                 in0=apx(X0, 32 + e1 * F0, (J0, CHI)),
                                 in1=apx(X0, 32 + e2 * F0, (J0, CHI)))
                    V.tensor_mul(out=apx(tBh, 0, (1, CHI)),
                                 in0=apx(X0, 32 + e3 * F0, (J0, CHI)),
                                 in1=apx(X0, 32 + e4 * F0, (J0, CHI)))
                    V.tensor_sub(out=apx(a32t, dsti, (12, CHI)),
                                 in0=apx(tAh, 0, (1, CHI)),
                                 in1=apx(tBh, 0, (1, CHI)))

                # cumsums (slab-contiguous fp16)
                for t in range(1, T0):
                    V.tensor_add(
                        out=apx(w0, t * 3 * F0, (1, 3 * F0)),
                        in0=apx(w0, t * 3 * F0, (1, 3 * F0)),
                        in1=apx(w0, (t - 1) * 3 * F0, (1, 3 * F0)))
                    if t < T1:
                        V.tensor_add(
                            out=apx(w1, t * 3 * F1, (1, 3 * F1)),
                            in0=apx(w1, t * 3 * F1, (1, 3 * F1)),
                            in1=apx(w1, (t - 1) * 3 * F1, (1, 3 * F1)))

                # a32 translation (slab 0 of cumsum = w slab 0)
                V.tensor_copy(out=apx(a32t, 3, (12, CHI), (4, 3)),
                              in_=apx(w0, 32, (J0, CHI), (F0, 3)))

                # block-total HTs -> fp32 packed bht
                def emit_bht(X, w, tW1, bht, F, T):
                    base = (T - 1) * 9 * F
                    SC.copy(out=apx(bht, 0, (12, F), (4, 2), (1, 3)),
                            in_=apx(X, base, (1, F), (3 * F, 2), (F, 3)))
                    SC.copy(out=apx(bht, 8, (12, F)),
                            in_=apx(tW1, (T - 1) * F, (1, F)))
                    for dsti, (e1, e2), (e3, e4) in ((9, (2, 3), (0, 5)),
                                                     (10, (0, 4), (1, 3))):
                        V.tensor_mul(out=apx(tAh, 0, (1, F)),
                                     in0=apx(X, base + e1 * F, (1, F)),
                                     in1=apx(X, base + e2 * F, (1, F)))
                        V.tensor_mul(out=apx(tBh, 0, (1, F)),
                                     in0=apx(X, base + e3 * F, (1, F)),
                                     in1=apx(X, base + e4 * F, (1, F)))
                        V.tensor_sub(out=apx(bht, dsti, (12, F)),
                                     in0=apx(tAh, 0, (1, F)),
                                     in1=apx(tBh, 0, (1, F)))
                    SC.copy(out=apx(bht, 3, (12, F), (4, 3)),
                            in_=apx(w, (T - 1) * 3 * F, (1, F), (F, 3)))

                emit_bht(X0, w0, tW1_0, bht0, F0, T0)
                emit_bht(X1, w1, tW1_1, bht1, F1, T1)

                # ---- hierarchy (fp32, as v1) ----
                LPS0 = (U0 + 1) * 12
                LPS1 = (U1 + 1) * 12
                V.tensor_copy(out=apx(lp2_0, 12, (LPS0, CHI * S0), (1, 12)),
                              in_=apx(bht0, 0, (U0 * 12, CHI * S0), (1, 12)))
                SC.copy(out=apx(lp2_1, 12, (LPS1, CHI * S1), (1, 12)),
                        in_=apx(bht1, 0, (U1 * 12, CHI * S1), (1, 12)))
                for u in range(1, U0):
                    compose_1d(V, CHI * S0,
                               a_off=u * 12, a_step=LPS0,
                               b_off=u * 12, b_step=U0 * 12,
                               o_off=(u + 1) * 12, o_step=LPS0,
                               tA=tAh, tB=tBh,
                               a_tile=lp2_0, b_tile=bht0, o_tile=lp2_0)
                    if u < U1:
                        compose_1d(V, CHI * S1,
                                   a_off=u * 12, a_step=LPS1,
                                   b_off=u * 12, b_step=U1 * 12,
                                   o_off=(u + 1) * 12, o_step=LPS1,
                                   tA=tA1h, tB=tB1h,
                                   a_tile=lp2_1, b_tile=bht1, o_tile=lp2_1)
                for sidx in range(1, S0):
                    compose_1d(V, CHI,
                               a_off=(sidx - 1) * 12, a_step=S0 * 12,
                               b_off=(sidx - 1) * LPS0 + U0 * 12,
                               b_step=S0 * LPS0,
                               o_off=sidx * 12, o_step=S0 * 12,
                               tA=tAh, tB=tBh,
                               a_tile=spx0, b_tile=lp2_0, o_tile=spx0)
                excl_blocks(V, SC, CHI * S0, U0, LPS0, spx0, lp2_0, rx0,
                            tAh, tBh)
                compose_1d(V, CHI,
                           a_off=32 * 12, a_step=J0 * 12,
                           b_off=0, b_step=12,
                           o_off=0, o_step=12,
                           tA=tAh, tB=tBh,
                           a_tile=rx0, b_tile=a32t, o_tile=rbr)
                SC.copy(out=apx(spx1, 0, (S1 * 12, CHI), (1, 12)),
                        in_=apx(rbr, 0, (12, CHI), (1, 12)))
                # rx -> planar fp16 for the down transform
                V.tensor_copy(out=apx(rx16_0, 0, (F0, 12), (1, F0)),
                              in_=apx(rx0, 0, (1, 12), (12, F0)))

                def down_i(w, rx16, X, tmpoff, F, T, i):
                    xyz = apx(X, 0, (3 * F, T), (F, 3), (1, F))
                    tmp = apx(X, tmpoff, (3 * F, T), (F, 3), (1, F))

                    def rxk(k):
                        return apx(rx16, k * F, (0, T), (4 * F, 3), (1, F))

                    def wk(k):
                        return apx(w, k * F, (3 * F, T), (0, 3), (1, F))

                    if i == 0:
                        V.tensor_mul(out=xyz, in0=rxk(0), in1=wk(0))
                    elif i == 1:
                        V.tensor_mul(out=tmp, in0=rxk(1), in1=wk(1))
                    elif i == 2:
                        V.tensor_add(out=xyz, in0=xyz, in1=tmp)
                    elif i == 3:
                        V.tensor_mul(out=tmp, in0=rxk(2), in1=wk(2))
                    elif i == 4:
                        V.tensor_add(out=xyz, in0=xyz, in1=tmp)
                    else:
                        V.tensor_add(out=xyz, in0=xyz, in1=rxk(3))

                # gen1 level-3 + excl first (covers the rx16_0 cast on ACT),
                # then the down-transforms; xyz stays planar for the DMA and
                # the host undoes the layout.
                for sidx in range(1, S1):
                    compose_1d(V, CHI,
                               a_off=(sidx - 1) * 12, a_step=S1 * 12,
                               b_off=(sidx - 1) * LPS1 + U1 * 12,
                               b_step=S1 * LPS1,
                               o_off=sidx * 12, o_step=S1 * 12,
                               tA=tA1h, tB=tB1h,
                               a_tile=spx1, b_tile=lp2_1, o_tile=spx1)
                excl_blocks(V, SC, CHI * S1, U1, LPS1, spx1, lp2_1, rx1,
                            tA1h, tB1h)
                V.tensor_copy(out=apx(rx16_1, 0, (F1, 12), (1, F1)),
                              in_=apx(rx1, 0, (1, 12), (12, F1)))
                for i in range(6):
                    down_i(w0, rx16_0, X0, 3 * F0 * T0, F0, T0, i)
                nc.sync.dma_start(
                    out=AP(kin0_d, 0, [[F0 * T0 * 3, P], [1, F0 * T0 * 3]]),
                    in_=apx(X0, 0, (1, F0 * T0 * 3)))
                for i in range(6):
                    down_i(w1, rx16_1, X1, 3 * F1 * T1, F1, T1, i)
                nc.sync.dma_start(
                    out=AP(kin1_d, 0, [[F1 * T1 * 3, P], [1, F1 * T1 * 3]]),
                    in_=apx(X1, 0, (1, F1 * T1 * 3)))

    nc.compile()
    return nc


def get_program(repeat=1):
    key = ("nc", repeat)
    if key not in _CACHE:
        _CACHE[key] = _build_program(repeat)
    return _CACHE[key]


# ------------------------------------------------------------------- host
def _shard_inputs(dofs, doftype):
    """Per-core inputs, pre-transposed to device t-major lane order.

    Device order per partition p: index t*F + chi*J + j for atom
    (chi, j, t); host layout [P, CHI*L].  Alpha is pre-folded on the host:
    alpha_p = phi_c(parent) + phi_p(p) (chain starts: phi_p only; branch
    roots fold phi_c of gen0 atom 384)."""
    def to_dev(arr, J, T):
        # arr: [C_core, L] (chain-major) -> [P, T*CHI*J]
        a = arr.reshape(CHI, P, J, T)
        return np.ascontiguousarray(
            a.transpose(1, 3, 0, 2).reshape(P, CHI * J * T))

    chain_starts = 1 + np.arange(C0, dtype=np.int64) * L0
    jd_all = np.ascontiguousarray(dofs[chain_starts])       # [C0, 9]

    ph0 = dofs[1:BOFF, 0].reshape(C0, L0)
    th0 = dofs[1:BOFF, 1].reshape(C0, L0)
    d0 = dofs[1:BOFF, 2].reshape(C0, L0)
    pc0 = dofs[1:BOFF, 3].reshape(C0, L0)
    al0 = np.empty_like(ph0)
    al0[:, 0] = 0.0
    al0[:, 1] = ph0[:, 1]
    al0[:, 2:] = ph0[:, 2:] + pc0[:, 1:-1]

    ph1 = dofs[BOFF:, 0].reshape(C1, L1)
    th1 = dofs[BOFF:, 1].reshape(C1, L1)
    d1 = dofs[BOFF:, 2].reshape(C1, L1)
    pc1 = dofs[BOFF:, 3].reshape(C1, L1)
    al1 = np.empty_like(ph1)
    al1[:, 0] = ph1[:, 0] + pc0[:, 384]
    al1[:, 1:] = ph1[:, 1:] + pc1[:, :-1]

    in_maps = []
    for core in range(NCORES):
        s0 = slice(core * CH0, (core + 1) * CH0)
        s1 = slice(core * CH1, (core + 1) * CH1)
        jd = np.ascontiguousarray(
            jd_all[s0].reshape(CHI, P, 9).transpose(1, 0, 2)
            .reshape(P, CHI * 9))
        in_maps.append({
            "th0": to_dev(th0[s0], J0, T0),
            "al0": to_dev(al0[s0], J0, T0),
            "dt0": to_dev(d0[s0], J0, T0),
            "th1": to_dev(th1[s1], J1, T1),
            "al1": to_dev(al1[s1], J1, T1),
            "dt1": to_dev(d1[s1], J1, T1),
            "jd": jd,
        })
    return in_maps


def _lane_ids(id_idx, core):
    """id_idx values of this core's atoms in device lane order (p, f, t)."""
    ids0 = (id_idx[core * A0:(core + 1) * A0]
            .reshape(CHI, P, L0).transpose(1, 0, 2).ravel())
    ids1 = (id_idx[BOFF - 1 + core * A1: BOFF - 1 + (core + 1) * A1]
            .reshape(CHI, P, L1).transpose(1, 0, 2).ravel())
    return ids0, ids1


def _structure_ok(doftype, gen0_paths, gen1_paths):
    chain_starts = 1 + np.arange(C0, dtype=np.int64) * L0
    g0 = np.concatenate(
        [np.zeros((C0, 1), np.int64), chain_starts[:, None] + np.arange(L0)],
        axis=1)
    if not np.array_equal(gen0_paths, g0.astype(gen0_paths.dtype)):
        return False
    branch_roots = chain_starts + L0 // 2
    g1 = np.concatenate(
        [branch_roots[:, None],
         BOFF + (np.arange(C1, dtype=np.int64) * L1)[:, None] + np.arange(L1)],
        axis=1)
    if not np.array_equal(gen1_paths, g1.astype(gen1_paths.dtype)):
        return False
    if doftype[0] != 0:
        return False
    if not np.all(doftype[chain_starts] == 1):
        return False
    dt = doftype.copy()
    dt[chain_starts] = 2
    if not np.all(dt[1:] == 2):
        return False
    return True


def _numpy_fallback(dofs, doftype, gen0_paths, gen1_paths, id_idx):
    """Exact numpy port of the reference (slow path, safety net)."""
    def rx(a):
        c, s = np.cos(a), np.sin(a)
        o, z = np.ones_like(a), np.zeros_like(a)
        return np.stack([np.stack([o, z, z, z], -1), np.stack([z, c, -s, z], -1),
                         np.stack([z, s, c, z], -1), np.stack([z, z, z, o], -1)], -2)

    def ry(a):
        c, s = np.cos(a), np.sin(a)
        o, z = np.ones_like(a), np.zeros_like(a)
        return np.stack([np.stack([c, z, s, z], -1), np.stack([z, o, z, z], -1),
                         np.stack([-s, z, c, z], -1), np.stack([z, z, z, o], -1)], -2)

    def rz(a):
        c, s = np.cos(a), np.sin(a)
        o, z = np.ones_like(a), np.zeros_like(a)
        return np.stack([np.stack([c, -s, z, z], -1), np.stack([s, c, z, z], -1),
                         np.stack([z, z, o, z], -1), np.stack([z, z, z, o], -1)], -2)

    def trans(x, y, z):
        o, zr = np.ones_like(x), np.zeros_like(x)
        return np.stack([np.stack([o, zr, zr, x], -1), np.stack([zr, o, zr, y], -1),
                         np.stack([zr, zr, o, z], -1), np.stack([zr, zr, zr, o], -1)], -2)

    dofs = dofs.astype(np.float32)
    phi_p, theta, d, phi_c = dofs[:, 0], dofs[:, 1], dofs[:, 2], dofs[:, 3]
    z = np.zeros_like(d)
    bond = rx(phi_p) @ rz(np.pi - theta) @ trans(d, z, z) @ rx(phi_c)
    rot = lambda a, b, c: rz(c) @ ry(b) @ rx(a)
    jump = (trans(dofs[:, 0], dofs[:, 1], dofs[:, 2])
            @ rot(dofs[:, 3], dofs[:, 4], dofs[:, 5])
            @ rot(dofs[:, 6], dofs[:, 7], dofs[:, 8]))
    eye = np.broadcast_to(np.eye(4, dtype=dofs.dtype), bond.shape)
    dt = doftype[:, None, None]
    hts = np.where(dt == 1, jump, np.where(dt == 2, bond, eye)).astype(np.float32)
    for paths in (gen0_paths, gen1_paths):
        seg = hts[paths]
        out = np.empty_like(seg)
        out[:, 0] = seg[:, 0]
        for i in range(1, seg.shape[1]):
            out[:, i] = out[:, i - 1] @ seg[:, i]
        hts[paths] = out
    kincoords = hts[:, :3, 3]
    coords = np.zeros((N - 1, 3), dtype=dofs.dtype)
    coords[np.asarray(id_idx)] = kincoords[1:]
    return coords


def kernel(dofs, doftype, gen0_paths, gen1_paths, id_idx):
    dofs = np.asarray(dofs, dtype=np.float32)
    doftype = np.asarray(doftype, dtype=np.int32)
    gen0_paths = np.asarray(gen0_paths)
    gen1_paths = np.asarray(gen1_paths)
    id_idx = np.asarray(id_idx, dtype=np.int32)

    if not _structure_ok(doftype, gen0_paths, gen1_paths):
        return _numpy_fallback(dofs, doftype, gen0_paths, gen1_paths, id_idx)

    from concourse.bass_utils import run_bass_kernel_spmd

    nc = get_program()
    in_maps = _shard_inputs(dofs, doftype)
    res = run_bass_kernel_spmd(nc, in_maps, core_ids=list(range(NCORES)))
    out = np.empty((N - 1, 3), dtype=np.float32)
    for core in range(NCORES):
        ids0, ids1 = _lane_ids(id_idx, core)
        k0 = res.results[core]["kin0"].astype(np.float32)
        k0 = k0.reshape(P, T0, 3, F0).transpose(0, 3, 1, 2).reshape(-1, 3)
        k1 = res.results[core]["kin1"].astype(np.float32)
        k1 = k1.reshape(P, T1, 3, F1).transpose(0, 3, 1, 2).reshape(-1, 3)
        out[ids0] = k0
        out[ids1] = k1
    return out
